# revision 2
# baseline (speedup 1.0000x reference)
"""Trainium2 Bass kernel for BalancedIPRMPNN (GNN message passing).

Reference computation (G=128 disjoint graphs, NPG=512 nodes each, H=128):
    h2   = x @ (W_emb @ W_gcn) + b_emb @ W_gcn          # embedding+GCN linear folded
    m    = relu(D^-1/2 (Adj + I) D^-1/2 @ h2 + b_gcn)   # GCN propagate (per graph)
    virt = einsum('gnv,gnh->gvh', edge_weights, m)      # weighted pooling (V=64)
    t1   = relu(virt @ vW1 + vb1)
    gf   = mean_v(t1 @ vW2 + vb2)
    out  = relu(gf @ mW1 + mb1) @ mW2 + mb2             # [G, 10]

Strategy: data-parallel over graphs, 16 graphs per core on 8 cores.  Message
passing runs as a dense per-graph [512,512] adjacency matmul on the tensor
engine (integer counts exact in fp8e4); the symmetric degree normalization is
folded into x (rows pre-scaled by dinv_src) and the pooling weights
(pre-scaled by dinv_dst, legal since relu commutes with a positive scale when
b_gcn == 0).

Fast path (edge_weights uniform across the virtual-node axis, zero biases —
the shipped init): virt rows are v-independent, so the whole virtual-node
block collapses to a single weighted node-sum per graph.  Kernel chain per
graph g:
    BT_g  = sum_kb  xs_blk^T @ Adj_blk         (PE, adj is the fp8 moving op)
    m_g   = BT_g^T @ W1                        (PE, via SBUF-cast of BT)
    mr_g  = relu(m_g)                          (ScalarE, one [128,512] pass)
    virtT[:, g] = sum_mb mr_blk^T @ cdinv_blk  (PE, weighted pooling)
then a tiny fp32 MLP tail over virtT [H, 16].  All bulk DMAs are issued
up-front into persistent SBUF tiles on two queues so the wire never idles,
and a few dummy matmuls warm the PE clock gate during the initial DMA wait.

Non-uniform edge_weights or nonzero biases fall back to the general program
(same math as the reference, dense per-graph pooling); non-disjoint graphs or
adjacency counts not exactly representable in fp8 fall back to numpy.
"""

import ml_dtypes
import numpy as np

import concourse.mybir as mybir
import concourse.tile as tile
from concourse import bacc
from concourse.bass_utils import run_bass_kernel_spmd

# Problem constants (hardcoded per contract)
G, NPG, H, IN, OUT, V = 128, 512, 128, 128, 10, 64
N = G * NPG
N_CORES = 8
GPC = G // N_CORES          # graphs per core = 16
NS = GPC * NPG              # nodes per core  = 8192
KB = NPG // 128             # 4 k-blocks of 128 nodes per graph

F32 = mybir.dt.float32
F16 = mybir.dt.float16
F8 = mybir.dt.float8e4

X_FP16 = True               # ship x in fp16 (halves x DMA, 4x faster h2 matmuls)

_CACHE = {}

# fp32 const-pack column offsets (fast path)
_C_VW1, _C_VW2, _C_MW1, _C_MW2 = 0, 128, 256, 384
_C_VB1, _C_VB2, _C_MB1, _C_MB2 = 394, 395, 396, 397
_C_W32 = 398


def _build_fast():
    """Fast-path program: uniform edge_weights, zero biases."""
    nc = bacc.Bacc("TRN2", target_bir_lowering=False)

    xp = nc.dram_tensor("xp", [128, GPC * KB * IN], F16, kind="ExternalInput")
    # adjacency counts (+I), 2 graphs per row: [j, p, gg*KB*NPG + kb*NPG + d]
    adjT = nc.dram_tensor("adjT", [GPC // 2, 128, 2 * KB * NPG], F8, kind="ExternalInput")
    # fp16 consts: W1 = W_emb @ W_gcn (cols 0:128), cdinv pooling vecs (128:128+GPC*KB)
    wf16 = nc.dram_tensor("wf16", [128, IN + GPC * KB], F16, kind="ExternalInput")
    # fp32 consts: vW1, vW2, mW1, mW2, then bias columns
    wf32 = nc.dram_tensor("wf32", [128, _C_W32], F32, kind="ExternalInput")
    outT = nc.dram_tensor("outT", [OUT, GPC], F32, kind="ExternalOutput")

    NQ = GPC // 4  # xp DMA chunks (4 graphs each)

    with tile.TileContext(nc) as tc:
        with (
            tc.tile_pool(name="consts", bufs=1) as consts,
            tc.tile_pool(name="btsb", bufs=3) as bt_pool,
            tc.tile_pool(name="mrel", bufs=3) as mr_pool,
            tc.tile_pool(name="pbt", bufs=2, space="PSUM") as pbt,
            tc.tile_pool(name="pm", bufs=2, space="PSUM") as pm,
            tc.tile_pool(name="pv", bufs=1, space="PSUM") as pv,
            tc.tile_pool(name="pwarm", bufs=1, space="PSUM") as pwarm,
            tc.tile_pool(name="ptail", bufs=2, space="PSUM") as ptail,
        ):
            # ---- all bulk DMAs up-front, two queues, critical data first ----
            wf16_sb = consts.tile([128, IN + GPC * KB], F16)
            nc.scalar.dma_start(out=wf16_sb[:], in_=wf16[:])
            xp_sb = consts.tile([128, GPC * KB * IN], F16)
            adj_sb = []
            for j in range(GPC // 2):
                a = consts.tile([128, 2 * KB * NPG], F8)
                nc.sync.dma_start(out=a[:], in_=adjT[j])
                adj_sb.append(a)
                if j < NQ:
                    cw = KB * IN * 4  # 4 graphs of xp per chunk
                    nc.scalar.dma_start(
                        out=xp_sb[:, j * cw:(j + 1) * cw],
                        in_=xp[:, j * cw:(j + 1) * cw],
                    )
            wf32_sb = consts.tile([128, _C_W32], F32)
            nc.scalar.dma_start(out=wf32_sb[:], in_=wf32[:])

            W1_sb = wf16_sb[:, 0:IN]
            cdinv = wf16_sb[:, IN:IN + GPC * KB]

            # ---- PE warm-up during the initial DMA wait ----
            scratch = consts.tile([128, NPG], F16)
            nc.vector.memset(scratch[:], 0.0)
            p_w = pwarm.tile([128, NPG], F32, tag="warm")
            for _ in range(6):
                nc.tensor.matmul(p_w[:], scratch[:, 0:128], scratch[:],
                                 start=True, stop=True)

            p_v = pv.tile([128, GPC], F32, tag="pv")

            bts, mps, mrs = {}, {}, {}

            def st1(g):
                j, gg = divmod(g, 2)
                p_bt = pbt.tile([128, NPG], F32, tag="bt")
                for kb in range(KB):
                    nc.tensor.matmul(
                        p_bt[:],
                        xp_sb[:, (g * KB + kb) * IN:(g * KB + kb + 1) * IN],
                        adj_sb[j][:, gg * KB * NPG + kb * NPG: gg * KB * NPG + (kb + 1) * NPG],
                        start=(kb == 0), stop=(kb == KB - 1),
                    )
                bt_sb = bt_pool.tile([128, NPG], F16, tag="btsb")
                nc.vector.tensor_copy(out=bt_sb[:], in_=p_bt[:])
                bts[g] = bt_sb

            def st2(g):
                p_m = pm.tile([128, KB * H], F32, tag="m")
                for mb in range(KB):
                    nc.tensor.matmul(
                        p_m[:, mb * H:(mb + 1) * H],
                        bts[g][:, mb * 128:(mb + 1) * 128],
                        W1_sb,
                        start=True, stop=True,
                    )
                mr = mr_pool.tile([128, KB * H], F16, tag="mr")
                nc.scalar.activation(
                    out=mr[:], in_=p_m[:],
                    func=mybir.ActivationFunctionType.Relu,
                )
                mrs[g] = mr
                del bts[g]

            def pool(g):
                for mb in range(KB):
                    nc.tensor.matmul(
                        p_v[:, g:g + 1],
                        mrs[g][:, mb * H:(mb + 1) * H],
                        cdinv[:, g * KB + mb:g * KB + mb + 1],
                        start=(mb == 0), stop=(mb == KB - 1),
                    )
                del mrs[g]

            for it in range(GPC + 3):
                if it < GPC:
                    st1(it)
                if 2 <= it < GPC + 2:
                    st2(it - 2)
                if it >= 3:
                    pool(it - 3)

            # ---- fp32 MLP tail over virtT [H, GPC] ----
            virtT = consts.tile([H, GPC], F32)
            nc.vector.tensor_copy(out=virtT[:], in_=p_v[:])
            p_t1 = ptail.tile([128, GPC], F32, tag="tail")
            nc.tensor.matmul(p_t1[:], wf32_sb[:, _C_VW1:_C_VW1 + H], virtT[:],
                             start=True, stop=True)
            t1 = consts.tile([H, GPC], F32)
            nc.scalar.activation(
                out=t1[:], in_=p_t1[:],
                func=mybir.ActivationFunctionType.Relu,
                bias=wf32_sb[:, _C_VB1:_C_VB1 + 1],
            )
            p_gf = ptail.tile([128, GPC], F32, tag="tail")
            nc.tensor.matmul(p_gf[:], wf32_sb[:, _C_VW2:_C_VW2 + H], t1[:],
                             start=True, stop=True)
            gf = consts.tile([H, GPC], F32)
            nc.scalar.activation(
                out=gf[:], in_=p_gf[:],
                func=mybir.ActivationFunctionType.Identity,
                bias=wf32_sb[:, _C_VB2:_C_VB2 + 1],
            )
            p_q1 = ptail.tile([128, GPC], F32, tag="tail")
            nc.tensor.matmul(p_q1[:], wf32_sb[:, _C_MW1:_C_MW1 + H], gf[:],
                             start=True, stop=True)
            q1 = consts.tile([H, GPC], F32)
            nc.scalar.activation(
                out=q1[:], in_=p_q1[:],
                func=mybir.ActivationFunctionType.Relu,
                bias=wf32_sb[:, _C_MB1:_C_MB1 + 1],
            )
            p_o = ptail.tile([OUT, GPC], F32, tag="tail")
            nc.tensor.matmul(p_o[:], wf32_sb[:, _C_MW2:_C_MW2 + OUT], q1[:],
                             start=True, stop=True)
            o_sb = consts.tile([OUT, GPC], F32)
            nc.scalar.activation(
                out=o_sb[:], in_=p_o[:],
                func=mybir.ActivationFunctionType.Identity,
                bias=wf32_sb[0:OUT, _C_MB2:_C_MB2 + 1],
            )
            nc.sync.dma_start(out=outT[:], in_=o_sb[:])

    nc.finalize()
    return nc


def _build_program(with_bias: bool):
    """General per-core program (any edge_weights / biases)."""
    nc = bacc.Bacc("TRN2", target_bir_lowering=False)
    XDT = F16 if X_FP16 else F32

    # ---- DRAM I/O ----
    xsT = nc.dram_tensor("xsT", [IN, NS], XDT, kind="ExternalInput")          # dinv-scaled x, transposed
    W1 = nc.dram_tensor("W1", [IN, H], XDT, kind="ExternalInput")             # W_emb @ W_gcn
    # adjacency counts (+I), pre-arranged to SBUF layout, 2 graphs per row:
    # [j, p, gg*KB*NPG + kb*NPG + d]  (gg in {0,1}, graph = 2j+gg)
    adjT = nc.dram_tensor("adjT", [GPC // 2, 128, 2 * KB * NPG], F8, kind="ExternalInput")
    # dinv-scaled edge_weights, pre-arranged likewise
    ews = nc.dram_tensor("ews", [GPC // 2, 128, 2 * KB * V], F16, kind="ExternalInput")
    vW1 = nc.dram_tensor("vW1", [H, H], F32, kind="ExternalInput")
    vb1 = nc.dram_tensor("vb1", [H, 1], F32, kind="ExternalInput")
    vW2s = nc.dram_tensor("vW2s", [H, H], F32, kind="ExternalInput")          # vW2 / V
    vb2 = nc.dram_tensor("vb2", [H, 1], F32, kind="ExternalInput")
    mW1 = nc.dram_tensor("mW1", [H, H], F32, kind="ExternalInput")
    mb1 = nc.dram_tensor("mb1", [H, 1], F32, kind="ExternalInput")
    mW2 = nc.dram_tensor("mW2", [H, OUT], F32, kind="ExternalInput")
    mb2 = nc.dram_tensor("mb2", [OUT, 1], F32, kind="ExternalInput")
    if with_bias:
        biasL = nc.dram_tensor("biasL", [GPC, 2, NPG], F16, kind="ExternalInput")
        biasR = nc.dram_tensor("biasR", [2, H], F16, kind="ExternalInput")
    outT = nc.dram_tensor("outT", [OUT, GPC], F32, kind="ExternalOutput")

    with tile.TileContext(nc) as tc:
        with (
            tc.tile_pool(name="consts", bufs=1) as consts,
            tc.tile_pool(name="xchunk", bufs=4) as xchunk_pool,
            tc.tile_pool(name="upool", bufs=3) as u_pool,
            tc.tile_pool(name="adj", bufs=4) as adj_pool,
            tc.tile_pool(name="ewsp", bufs=4) as ews_pool,
            tc.tile_pool(name="mp", bufs=3) as m_pool,
            tc.tile_pool(name="blp", bufs=3) as bl_pool,
            tc.tile_pool(name="ph2", bufs=2, space="PSUM") as ph2,
            tc.tile_pool(name="pm", bufs=4, space="PSUM") as pm,
            tc.tile_pool(name="pv", bufs=1, space="PSUM") as pv,
            tc.tile_pool(name="pd", bufs=1, space="PSUM") as pd,
        ):
            # critical-path data first: graph pair 0's inputs, then W1
            xc0 = xchunk_pool.tile([IN, 2 * NPG], XDT, tag="xc")
            nc.sync.dma_start(out=xc0[:], in_=xsT[:, 0:2 * NPG])
            W1_sb = consts.tile([IN, H], XDT)
            nc.sync.dma_start(out=W1_sb[:], in_=W1[:])
            adj0 = adj_pool.tile([128, 2 * KB * NPG], F8, tag="adj")
            nc.sync.dma_start(out=adj0[:], in_=adjT[0])
            ews0 = ews_pool.tile([128, 2 * KB * V], F16, tag="ews")
            nc.sync.dma_start(out=ews0[:], in_=ews[0])

            vW1_sb = consts.tile([H, H], F32)
            nc.scalar.dma_start(out=vW1_sb[:], in_=vW1[:])
            vW2_sb = consts.tile([H, H], F32)
            nc.scalar.dma_start(out=vW2_sb[:], in_=vW2s[:])
            mW1_sb = consts.tile([H, H], F32)
            nc.scalar.dma_start(out=mW1_sb[:], in_=mW1[:])
            mW2_sb = consts.tile([H, OUT], F32)
            nc.scalar.dma_start(out=mW2_sb[:], in_=mW2[:])
            vb1_sb = consts.tile([H, 1], F32)
            nc.scalar.dma_start(out=vb1_sb[:], in_=vb1[:])
            vb2_sb = consts.tile([H, 1], F32)
            nc.scalar.dma_start(out=vb2_sb[:], in_=vb2[:])
            mb1_sb = consts.tile([H, 1], F32)
            nc.scalar.dma_start(out=mb1_sb[:], in_=mb1[:])
            mb2_sb = consts.tile([OUT, 1], F32)
            nc.scalar.dma_start(out=mb2_sb[:], in_=mb2[:])
            if with_bias:
                biasR_sb = consts.tile([2, H], F16)
                nc.scalar.dma_start(out=biasR_sb[:], in_=biasR[:])

            virtT = consts.tile([H, GPC * V], F32)  # virt^T, all graphs side by side
            t1 = consts.tile([H, GPC * V], F32)
            t1s = consts.tile([H, GPC], F32)

            def emit_embed(j):
                # u = (dinv*x) @ W1, cast fp16, for graph pair j (1024 nodes)
                if j == 0:
                    xc = xc0
                else:
                    xc = xchunk_pool.tile([IN, 2 * NPG], XDT, tag="xc")
                    nc.sync.dma_start(out=xc[:], in_=xsT[:, 2 * j * NPG:2 * (j + 1) * NPG])
                u_j = u_pool.tile([128, 2 * KB * H], F16, tag="u")
                for half in range(2):
                    p_h2 = ph2.tile([128, KB * H], F32, tag="ph2")
                    for kb in range(KB):
                        nc.tensor.matmul(
                            p_h2[:, kb * H:(kb + 1) * H],
                            xc[:, half * NPG + kb * 128: half * NPG + (kb + 1) * 128],
                            W1_sb[:],
                            start=True, stop=True,
                        )
                    nc.vector.tensor_copy(
                        out=u_j[:, half * KB * H:(half + 1) * KB * H], in_=p_h2[:])
                return u_j

            us = [emit_embed(0)]
            pending = []
            for g in range(GPC):
                j, gg = divmod(g, 2)
                if gg == 0:
                    if j + 1 < GPC // 2:
                        us.append(emit_embed(j + 1))
                    if j == 0:
                        adj_pair, ews_pair = adj0, ews0
                    else:
                        adj_pair = adj_pool.tile([128, 2 * KB * NPG], F8, tag="adj")
                        nc.sync.dma_start(out=adj_pair[:], in_=adjT[j])
                        ews_pair = ews_pool.tile([128, 2 * KB * V], F16, tag="ews")
                        nc.sync.dma_start(out=ews_pair[:], in_=ews[j])
                u_g = us[j][:, gg * KB * H:(gg + 1) * KB * H]
                adj_sb = adj_pair[:, gg * KB * NPG:(gg + 1) * KB * NPG]
                ews_sb = ews_pair[:, gg * KB * V:(gg + 1) * KB * V]
                if with_bias:
                    bl_sb = bl_pool.tile([2, NPG], F16, tag="bl")
                    nc.sync.dma_start(out=bl_sb[:], in_=biasL[g])

                m_sb = m_pool.tile([128, KB * H], F16, tag="m")
                for mb in range(KB):
                    p_m = pm.tile([128, H], F32, tag="pm")
                    if with_bias:
                        nc.tensor.matmul(
                            p_m[:], bl_sb[:, mb * 128:(mb + 1) * 128], biasR_sb[:],
                            start=True, stop=False,
                        )
                    for kb in range(KB):
                        nc.tensor.matmul(
                            p_m[:],
                            adj_sb[:, kb * NPG + mb * 128: kb * NPG + (mb + 1) * 128],
                            u_g[:, kb * H:(kb + 1) * H],
                            start=(kb == 0 and not with_bias),
                            stop=(kb == KB - 1),
                        )
                    nc.scalar.activation(
                        out=m_sb[:, mb * H:(mb + 1) * H], in_=p_m[:],
                        func=mybir.ActivationFunctionType.Relu,
                    )

                # ---- pooling (deferred by one graph so the relu is long done
                # by the time the PE reaches these matmuls) ----
                pending.append((g, m_sb, ews_sb))
                emit_g = g - 1 if g < GPC - 1 else None
                ready = [p for p in pending if p[0] == emit_g]
                if g == GPC - 1:
                    ready = list(pending)
                for eg, e_m, e_ews in ready:
                    p_v = pv.tile([128, V], F32, tag="pv")
                    for kb in range(KB):
                        nc.tensor.matmul(
                            p_v[:],
                            e_m[:, kb * H:(kb + 1) * H],
                            e_ews[:, kb * V:(kb + 1) * V],
                            start=(kb == 0), stop=(kb == KB - 1),
                        )
                    nc.vector.tensor_copy(out=virtT[:, eg * V:(eg + 1) * V], in_=p_v[:])
                    pending.remove((eg, e_m, e_ews))

                # ---- MLP first stage per quarter once its 4 graphs are emitted ----
                for q in range(4):
                    if g != (4 * q + 5 if q < 3 else GPC - 1):
                        continue
                    p_t1 = pd.tile([128, 256], F32, tag="pd")
                    nc.tensor.matmul(
                        p_t1[:], vW1_sb[:], virtT[:, q * 256:(q + 1) * 256],
                        start=True, stop=True,
                    )
                    nc.scalar.activation(
                        out=t1[:, q * 256:(q + 1) * 256], in_=p_t1[:],
                        func=mybir.ActivationFunctionType.Relu, bias=vb1_sb[:],
                    )
                    nc.vector.tensor_reduce(
                        out=t1s[:, q * 4:(q + 1) * 4],
                        in_=t1[:, q * 256:(q + 1) * 256]
                            .rearrange("p (g v) -> p g v", v=V),
                        axis=mybir.AxisListType.X, op=mybir.AluOpType.add,
                    )

            # ---- rest of the MLP tail ----
            p_gf = pd.tile([128, GPC], F32, tag="pd")
            nc.tensor.matmul(p_gf[:], vW2_sb[:], t1s[:], start=True, stop=True)
            gf = consts.tile([H, GPC], F32)
            nc.scalar.activation(
                out=gf[:], in_=p_gf[:],
                func=mybir.ActivationFunctionType.Identity, bias=vb2_sb[:],
            )
            p_q1 = pd.tile([128, GPC], F32, tag="pd")
            nc.tensor.matmul(p_q1[:], mW1_sb[:], gf[:], start=True, stop=True)
            q1 = consts.tile([H, GPC], F32)
            nc.scalar.activation(
                out=q1[:], in_=p_q1[:],
                func=mybir.ActivationFunctionType.Relu, bias=mb1_sb[:],
            )
            p_o = pd.tile([OUT, GPC], F32, tag="pd")
            nc.tensor.matmul(p_o[:], mW2_sb[:], q1[:], start=True, stop=True)
            o_sb = consts.tile([OUT, GPC], F32)
            nc.scalar.activation(
                out=o_sb[:], in_=p_o[:],
                func=mybir.ActivationFunctionType.Identity, bias=mb2_sb[:],
            )
            nc.sync.dma_start(out=outT[:], in_=o_sb[:])

    nc.finalize()
    return nc


def _reference_numpy(x, edge_index, W_emb, b_emb, W_gcn, b_gcn, edge_weights,
                     vW1, vb1, vW2, vb2, mW1, mb1, mW2, mb2):
    """Pure-numpy fallback (used only if graphs are not disjoint)."""
    src, dst = edge_index[0].astype(np.int64), edge_index[1].astype(np.int64)
    h = x @ W_emb + b_emb
    h2 = h @ W_gcn
    deg = np.bincount(dst, minlength=N).astype(np.float32) + 1.0
    dinv = 1.0 / np.sqrt(deg)
    m = np.zeros_like(h2)
    np.add.at(m, dst, h2[src] * (dinv[src] * dinv[dst])[:, None])
    m += h2 * (dinv * dinv)[:, None]
    m = np.maximum(m + b_gcn, 0.0)
    hg = m.reshape(G, NPG, -1)
    virt = np.einsum('gnv,gnh->gvh', edge_weights, hg)
    t1 = np.maximum(virt @ vW1 + vb1, 0.0) @ vW2 + vb2
    gf = t1.mean(axis=1)
    return np.maximum(gf @ mW1 + mb1, 0.0) @ mW2 + mb2


def kernel(x, edge_index, batch, W_emb, b_emb, W_gcn, b_gcn, edge_weights,
           vW1, vb1, vW2, vb2, mW1, mb1, mW2, mb2, _trace=False):
    x = np.asarray(x, dtype=np.float32)
    edge_index = np.asarray(edge_index, dtype=np.int32)
    W_emb = np.asarray(W_emb, dtype=np.float32)
    b_emb = np.asarray(b_emb, dtype=np.float32)
    W_gcn = np.asarray(W_gcn, dtype=np.float32)
    b_gcn = np.asarray(b_gcn, dtype=np.float32)
    edge_weights = np.asarray(edge_weights, dtype=np.float32)
    vW1, vb1 = np.asarray(vW1, np.float32), np.asarray(vb1, np.float32)
    vW2, vb2 = np.asarray(vW2, np.float32), np.asarray(vb2, np.float32)
    mW1, mb1 = np.asarray(mW1, np.float32), np.asarray(mb1, np.float32)
    mW2, mb2 = np.asarray(mW2, np.float32), np.asarray(mb2, np.float32)

    src = edge_index[0].astype(np.int64)
    dst = edge_index[1].astype(np.int64)
    if not np.array_equal(src // NPG, dst // NPG):
        # cross-graph edges: dense per-graph adjacency doesn't apply
        return _reference_numpy(x, edge_index, W_emb, b_emb, W_gcn, b_gcn,
                                edge_weights, vW1, vb1, vW2, vb2, mW1, mb1,
                                mW2, mb2).astype(np.float32)

    # ---- host prep ----
    deg = (np.bincount(dst, minlength=N) + 1).astype(np.float32)  # in-degree + self loop
    dinv = (1.0 / np.sqrt(deg)).astype(np.float32)

    # per-graph transposed adjacency counts (+ self loops), exact small ints in fp8e4
    gidx = src // NPG
    lin = (gidx * NPG + (src % NPG)) * NPG + (dst % NPG)
    counts = np.bincount(lin, minlength=G * NPG * NPG)
    adjT_all = counts.reshape(G, NPG, NPG).astype(np.float32)
    diag = np.arange(NPG)
    adjT_all[:, diag, diag] += np.float32(1.0)
    if adjT_all.max() > 16:  # not exactly representable in fp8e4
        return _reference_numpy(x, edge_index, W_emb, b_emb, W_gcn, b_gcn,
                                edge_weights, vW1, vb1, vW2, vb2, mW1, mb1,
                                mW2, mb2).astype(np.float32)
    adjT_all = adjT_all.astype(ml_dtypes.float8_e4m3)
    # SBUF layout: [g, p, kb*NPG + d], then merge graph pairs so each DMA is
    # one [128, contiguous] block covering 2 graphs
    adjT_sb_all = (
        adjT_all.reshape(G, KB, 128, NPG).transpose(0, 2, 1, 3).reshape(G, 128, KB * NPG)
    )
    adjT_sb_all = np.ascontiguousarray(
        adjT_sb_all.reshape(G // 2, 2, 128, KB * NPG).transpose(0, 2, 1, 3)
        .reshape(G // 2, 128, 2 * KB * NPG)
    )

    bvec = (b_emb @ W_gcn).astype(np.float32)
    with_bias = bool(np.any(bvec) or np.any(b_gcn))
    ew_col = edge_weights[:, :, 0]
    uniform = bool(np.all(edge_weights == ew_col[:, :, None]))
    W1h = (W_emb @ W_gcn).astype(np.float16)

    if uniform and not with_bias:
        # ---- fast path ----
        xs = (x * dinv[:, None]).astype(np.float16)      # fold D^-1/2_src into x
        # pooling weights: edge_weights column * dinv_dst
        cd = (ew_col * dinv.reshape(G, NPG)).astype(np.float16)  # [G, NPG]

        wf32_np = np.zeros((128, _C_W32), np.float32)
        wf32_np[:, _C_VW1:_C_VW1 + H] = vW1
        wf32_np[:, _C_VW2:_C_VW2 + H] = vW2
        wf32_np[:, _C_MW1:_C_MW1 + H] = mW1
        wf32_np[:, _C_MW2:_C_MW2 + OUT] = mW2
        wf32_np[:, _C_VB1] = vb1
        wf32_np[:, _C_VB2] = vb2
        wf32_np[:, _C_MB1] = mb1
        wf32_np[:OUT, _C_MB2] = mb2

        if "fast" not in _CACHE:
            _CACHE["fast"] = _build_fast()
        nc = _CACHE["fast"]

        in_maps = []
        for c in range(N_CORES):
            xs_c = xs[c * NS:(c + 1) * NS]  # [8192, 128]
            xp_np = np.ascontiguousarray(
                xs_c.reshape(GPC * KB, 128, IN).transpose(1, 0, 2)
                .reshape(128, GPC * KB * IN)
            )
            cd_c = cd[c * GPC:(c + 1) * GPC]  # [GPC, NPG]
            cdp = np.ascontiguousarray(
                cd_c.reshape(GPC, KB, 128).transpose(2, 0, 1).reshape(128, GPC * KB)
            )
            wf16_np = np.concatenate([W1h, cdp], axis=1)  # [128, IN + GPC*KB]
            ps = slice(c * GPC // 2, (c + 1) * GPC // 2)
            in_maps.append({
                "xp": xp_np,
                "adjT": adjT_sb_all[ps],
                "wf16": np.ascontiguousarray(wf16_np),
                "wf32": wf32_np,
            })
    else:
        # ---- general path ----
        xdt = np.float16 if X_FP16 else np.float32
        xs = (x * dinv[:, None])  # fold D^-1/2 into x rows
        xsT_np = np.ascontiguousarray(xs.T.astype(xdt))  # [IN, N]
        ews_all = (edge_weights * dinv.reshape(G, NPG, 1)).astype(np.float16)
        ews_sb_all = (
            ews_all.reshape(G, KB, 128, V).transpose(0, 2, 1, 3).reshape(G, 128, KB * V)
        )
        ews_sb_all = np.ascontiguousarray(
            ews_sb_all.reshape(G // 2, 2, 128, KB * V).transpose(0, 2, 1, 3)
            .reshape(G // 2, 128, 2 * KB * V)
        )

        vW2s_h = (vW2 / np.float32(V)).astype(np.float32)
        if with_bias:
            # m-psum bias = wvec ⊗ bvec + sqrt(deg) ⊗ b_gcn, with
            # wvec = (Adj+I) @ dinv per graph (host-computable rank-2 correction)
            dinv_g = dinv.reshape(G, NPG)
            wvec = np.einsum('gsd,gs->gd', adjT_all.astype(np.float32), dinv_g)
            sdeg = np.sqrt(deg).reshape(G, NPG)
            biasL_all = np.stack([wvec, sdeg], axis=1).astype(np.float16)  # [G, 2, NPG]
            biasR_np = np.stack([bvec, b_gcn], axis=0).astype(np.float16)  # [2, H]

        key = with_bias
        if key not in _CACHE:
            _CACHE[key] = _build_program(with_bias)
        nc = _CACHE[key]

        in_maps = []
        for c in range(N_CORES):
            gs = slice(c * GPC, (c + 1) * GPC)
            ps = slice(c * GPC // 2, (c + 1) * GPC // 2)
            im = {
                "xsT": np.ascontiguousarray(xsT_np[:, c * NS:(c + 1) * NS]),
                "W1": W1h if X_FP16 else (W_emb @ W_gcn).astype(np.float32),
                "adjT": adjT_sb_all[ps],
                "ews": ews_sb_all[ps],
                "vW1": vW1, "vb1": vb1.reshape(H, 1),
                "vW2s": vW2s_h, "vb2": vb2.reshape(H, 1),
                "mW1": mW1, "mb1": mb1.reshape(H, 1),
                "mW2": mW2, "mb2": mb2.reshape(OUT, 1),
            }
            if with_bias:
                im["biasL"] = np.ascontiguousarray(biasL_all[gs])
                im["biasR"] = biasR_np
            in_maps.append(im)

    res = run_bass_kernel_spmd(
        nc, in_maps, core_ids=list(range(N_CORES)), trace=_trace,
    )
    out = np.concatenate([res.results[c]["outT"].T for c in range(N_CORES)], axis=0)
    if _trace:
        kernel.last_exec_time_ns = res.exec_time_ns
        kernel.last_results = res
    return out.astype(np.float32)


# revision 6
# speedup vs baseline: 1.5935x; 1.5935x over previous
"""Trainium2 Bass kernel for BalancedIPRMPNN (GNN message passing).

Reference computation (G=128 disjoint graphs, NPG=512 nodes each, H=128):
    h2   = x @ (W_emb @ W_gcn) + b_emb @ W_gcn          # embedding+GCN linear folded
    m    = relu(D^-1/2 (Adj + I) D^-1/2 @ h2 + b_gcn)   # GCN propagate (per graph)
    virt = einsum('gnv,gnh->gvh', edge_weights, m)      # weighted pooling (V=64)
    t1   = relu(virt @ vW1 + vb1)
    gf   = mean_v(t1 @ vW2 + vb2)
    out  = relu(gf @ mW1 + mb1) @ mW2 + mb2             # [G, 10]

Strategy: data-parallel over graphs, 16 graphs per core on 8 cores.  Message
passing runs as a dense per-graph [512,512] adjacency matmul on the tensor
engine (integer counts exact in fp8e4); the symmetric degree normalization is
folded into x (rows pre-scaled by dinv_src) and the pooling weights
(pre-scaled by dinv_dst, legal since relu commutes with a positive scale when
b_gcn == 0).

Fast path (edge_weights uniform across the virtual-node axis, zero biases —
the shipped init): virt rows are v-independent, so the whole virtual-node
block collapses to a single weighted node-sum per graph.  Kernel chain per
graph g:
    BT_g  = sum_kb  xs_blk^T @ Adj_blk         (PE, adj is the fp8 moving op)
    m_g   = BT_g^T @ W1                        (PE, via SBUF-cast of BT)
    mr_g  = relu(m_g)                          (ScalarE, one [128,512] pass)
    virtT[:, g] = sum_mb mr_blk^T @ cdinv_blk  (PE, weighted pooling)
then a tiny fp32 MLP tail over virtT [H, 16].  All bulk DMAs are issued
up-front into persistent SBUF tiles on two queues so the wire never idles,
and a few dummy matmuls warm the PE clock gate during the initial DMA wait.

Non-uniform edge_weights or nonzero biases fall back to the general program
(same math as the reference, dense per-graph pooling); non-disjoint graphs or
adjacency counts not exactly representable in fp8 fall back to numpy.
"""

import ml_dtypes
import numpy as np

import concourse.mybir as mybir
import concourse.tile as tile
from concourse import bacc
from concourse.bass_utils import run_bass_kernel_spmd

# Problem constants (hardcoded per contract)
G, NPG, H, IN, OUT, V = 128, 512, 128, 128, 10, 64
N = G * NPG
N_CORES = 8
GPC = G // N_CORES          # graphs per core = 16
NS = GPC * NPG              # nodes per core  = 8192
KB = NPG // 128             # 4 k-blocks of 128 nodes per graph

F32 = mybir.dt.float32
F16 = mybir.dt.float16
F8 = mybir.dt.float8e4

X_FP16 = True               # ship x in fp16 (halves x DMA, 4x faster h2 matmuls)

_CACHE = {}

# fp32 const-pack column offsets (fast path)
_C_VW1, _C_VW2, _C_MW1, _C_MW2 = 0, 128, 256, 384
_C_VB1, _C_VB2, _C_MB1, _C_MB2 = 394, 395, 396, 397
_C_W32 = 398


def _build_fast():
    """Fast-path program: uniform edge_weights, zero biases."""
    nc = bacc.Bacc("TRN2", target_bir_lowering=False)

    PR = 2 * KB * NPG  # adjacency columns per graph pair (4096)

    xp = nc.dram_tensor("xp", [128, GPC * KB * IN], F16, kind="ExternalInput")
    # adjacency counts (+I), quad-merged: [q, p, jj*PR + gg*KB*NPG + kb*NPG + d]
    adjT = nc.dram_tensor("adjT", [GPC // 4, 128, 2 * PR], F8, kind="ExternalInput")
    # fp16 consts: W1 = W_emb @ W_gcn (cols 0:128), cdinv pooling vecs (128:128+GPC*KB)
    wf16 = nc.dram_tensor("wf16", [128, IN + GPC * KB], F16, kind="ExternalInput")
    # fp32 consts: vW1, vW2, mW1, mW2, then bias columns
    wf32 = nc.dram_tensor("wf32", [128, _C_W32], F32, kind="ExternalInput")
    outT = nc.dram_tensor("outT", [OUT, GPC], F32, kind="ExternalOutput")

    with tile.TileContext(nc) as tc:
        with (
            tc.tile_pool(name="consts", bufs=1) as consts,
            tc.tile_pool(name="btsb", bufs=3) as bt_pool,
            tc.tile_pool(name="mrel", bufs=3) as mr_pool,
            tc.tile_pool(name="pbt", bufs=2, space="PSUM") as pbt,
            tc.tile_pool(name="pm", bufs=2, space="PSUM") as pm,
            tc.tile_pool(name="pv", bufs=1, space="PSUM") as pv,
            tc.tile_pool(name="pwarm", bufs=1, space="PSUM") as pwarm,
            tc.tile_pool(name="ptail", bufs=2, space="PSUM") as ptail,
        ):
            # ---- bulk DMA plan -------------------------------------------
            # Consts + first xp chunk ride the gpsimd SWDGE queue (its own
            # semaphore pool); the bulk stream runs on the sync HWDGE queue
            # in consumption order.  Keeping the sync queue at <= 8
            # outstanding semaphore uses avoids the framework's sem-reuse
            # guards, which otherwise pace each DMA on the consumers of an
            # earlier one (the v1 failure mode: a 45us trickle).
            wf16_sb = consts.tile([128, IN + GPC * KB], F16)
            nc.gpsimd.dma_start(out=wf16_sb[:], in_=wf16[:])
            xp_sb = consts.tile([128, GPC * KB * IN], F16)
            adj_all = consts.tile([128, (GPC // 2) * PR], F8)

            XC = KB * IN * 4  # xp columns per 4-graph chunk (2048)

            def xp_dma(q, eng):
                eng.dma_start(out=xp_sb[:, q * XC:(q + 1) * XC],
                              in_=xp[:, q * XC:(q + 1) * XC])

            def adj_dma(lo, hi):  # pairs [lo, hi) as one DMA (within one quad)
                assert lo // 2 == (hi - 1) // 2
                nc.sync.dma_start(
                    out=adj_all[:, lo * PR:hi * PR],
                    in_=adjT[lo // 2, :, (lo % 2) * PR:(lo % 2 + hi - lo) * PR],
                )

            adj_dma(0, 1)
            xp_dma(0, nc.gpsimd)
            wf32_sb = consts.tile([128, _C_W32], F32)
            nc.gpsimd.dma_start(out=wf32_sb[:], in_=wf32[:])
            adj_dma(1, 2)
            adj_dma(2, 3)
            xp_dma(1, nc.sync)
            adj_dma(3, 4)
            adj_dma(4, 6)
            xp_dma(2, nc.sync)
            adj_dma(6, 8)
            xp_dma(3, nc.sync)

            W1_sb = wf16_sb[:, 0:IN]
            cdinv = wf16_sb[:, IN:IN + GPC * KB]

            # ---- PE warm-up during the initial DMA wait ----
            scratch = consts.tile([128, NPG], F16)
            nc.vector.memset(scratch[:], 0.0)
            p_w = pwarm.tile([128, NPG], F32, tag="warm")
            for _ in range(6):
                nc.tensor.matmul(p_w[:], scratch[:, 0:128], scratch[:],
                                 start=True, stop=True)

            p_v = pv.tile([128, GPC], F32, tag="pv")

            bts, mps, mrs = {}, {}, {}

            def st1(g):
                j, gg = divmod(g, 2)
                base = j * PR + gg * KB * NPG
                p_bt = pbt.tile([128, NPG], F32, tag="bt")
                for kb in range(KB):
                    nc.tensor.matmul(
                        p_bt[:],
                        xp_sb[:, (g * KB + kb) * IN:(g * KB + kb + 1) * IN],
                        adj_all[:, base + kb * NPG: base + (kb + 1) * NPG],
                        start=(kb == 0), stop=(kb == KB - 1),
                    )
                bt_sb = bt_pool.tile([128, NPG], F16, tag="btsb")
                nc.vector.tensor_copy(out=bt_sb[:], in_=p_bt[:])
                bts[g] = bt_sb

            def st2(g):
                p_m = pm.tile([128, KB * H], F32, tag="m")
                for mb in range(KB):
                    nc.tensor.matmul(
                        p_m[:, mb * H:(mb + 1) * H],
                        bts[g][:, mb * 128:(mb + 1) * 128],
                        W1_sb,
                        start=True, stop=True,
                    )
                mr = mr_pool.tile([128, KB * H], F16, tag="mr")
                nc.scalar.activation(
                    out=mr[:], in_=p_m[:],
                    func=mybir.ActivationFunctionType.Relu,
                )
                mrs[g] = mr
                del bts[g]

            def pool(g):
                for mb in range(KB):
                    nc.tensor.matmul(
                        p_v[:, g:g + 1],
                        mrs[g][:, mb * H:(mb + 1) * H],
                        cdinv[:, g * KB + mb:g * KB + mb + 1],
                        start=(mb == 0), stop=(mb == KB - 1),
                    )
                del mrs[g]

            for it in range(GPC + 3):
                if it < GPC:
                    st1(it)
                if 2 <= it < GPC + 2:
                    st2(it - 2)
                if it >= 3:
                    pool(it - 3)

            # ---- fp32 MLP tail over virtT [H, GPC] ----
            virtT = consts.tile([H, GPC], F32)
            nc.vector.tensor_copy(out=virtT[:], in_=p_v[:])
            p_t1 = ptail.tile([128, GPC], F32, tag="tail")
            nc.tensor.matmul(p_t1[:], wf32_sb[:, _C_VW1:_C_VW1 + H], virtT[:],
                             start=True, stop=True)
            t1 = consts.tile([H, GPC], F32)
            nc.scalar.activation(
                out=t1[:], in_=p_t1[:],
                func=mybir.ActivationFunctionType.Relu,
                bias=wf32_sb[:, _C_VB1:_C_VB1 + 1],
            )
            p_gf = ptail.tile([128, GPC], F32, tag="tail")
            nc.tensor.matmul(p_gf[:], wf32_sb[:, _C_VW2:_C_VW2 + H], t1[:],
                             start=True, stop=True)
            gf = consts.tile([H, GPC], F32)
            nc.scalar.activation(
                out=gf[:], in_=p_gf[:],
                func=mybir.ActivationFunctionType.Identity,
                bias=wf32_sb[:, _C_VB2:_C_VB2 + 1],
            )
            p_q1 = ptail.tile([128, GPC], F32, tag="tail")
            nc.tensor.matmul(p_q1[:], wf32_sb[:, _C_MW1:_C_MW1 + H], gf[:],
                             start=True, stop=True)
            q1 = consts.tile([H, GPC], F32)
            nc.scalar.activation(
                out=q1[:], in_=p_q1[:],
                func=mybir.ActivationFunctionType.Relu,
                bias=wf32_sb[:, _C_MB1:_C_MB1 + 1],
            )
            p_o = ptail.tile([OUT, GPC], F32, tag="tail")
            nc.tensor.matmul(p_o[:], wf32_sb[:, _C_MW2:_C_MW2 + OUT], q1[:],
                             start=True, stop=True)
            o_sb = consts.tile([OUT, GPC], F32)
            nc.scalar.activation(
                out=o_sb[:], in_=p_o[:],
                func=mybir.ActivationFunctionType.Identity,
                bias=wf32_sb[0:OUT, _C_MB2:_C_MB2 + 1],
            )
            nc.sync.dma_start(out=outT[:], in_=o_sb[:])

    nc.finalize()
    return nc


def _build_program(with_bias: bool):
    """General per-core program (any edge_weights / biases)."""
    nc = bacc.Bacc("TRN2", target_bir_lowering=False)
    XDT = F16 if X_FP16 else F32

    # ---- DRAM I/O ----
    xsT = nc.dram_tensor("xsT", [IN, NS], XDT, kind="ExternalInput")          # dinv-scaled x, transposed
    W1 = nc.dram_tensor("W1", [IN, H], XDT, kind="ExternalInput")             # W_emb @ W_gcn
    # adjacency counts (+I), pre-arranged to SBUF layout, 2 graphs per row:
    # [j, p, gg*KB*NPG + kb*NPG + d]  (gg in {0,1}, graph = 2j+gg)
    adjT = nc.dram_tensor("adjT", [GPC // 2, 128, 2 * KB * NPG], F8, kind="ExternalInput")
    # dinv-scaled edge_weights, pre-arranged likewise
    ews = nc.dram_tensor("ews", [GPC // 2, 128, 2 * KB * V], F16, kind="ExternalInput")
    vW1 = nc.dram_tensor("vW1", [H, H], F32, kind="ExternalInput")
    vb1 = nc.dram_tensor("vb1", [H, 1], F32, kind="ExternalInput")
    vW2s = nc.dram_tensor("vW2s", [H, H], F32, kind="ExternalInput")          # vW2 / V
    vb2 = nc.dram_tensor("vb2", [H, 1], F32, kind="ExternalInput")
    mW1 = nc.dram_tensor("mW1", [H, H], F32, kind="ExternalInput")
    mb1 = nc.dram_tensor("mb1", [H, 1], F32, kind="ExternalInput")
    mW2 = nc.dram_tensor("mW2", [H, OUT], F32, kind="ExternalInput")
    mb2 = nc.dram_tensor("mb2", [OUT, 1], F32, kind="ExternalInput")
    if with_bias:
        biasL = nc.dram_tensor("biasL", [GPC, 2, NPG], F16, kind="ExternalInput")
        biasR = nc.dram_tensor("biasR", [2, H], F16, kind="ExternalInput")
    outT = nc.dram_tensor("outT", [OUT, GPC], F32, kind="ExternalOutput")

    with tile.TileContext(nc) as tc:
        with (
            tc.tile_pool(name="consts", bufs=1) as consts,
            tc.tile_pool(name="xchunk", bufs=4) as xchunk_pool,
            tc.tile_pool(name="upool", bufs=3) as u_pool,
            tc.tile_pool(name="adj", bufs=4) as adj_pool,
            tc.tile_pool(name="ewsp", bufs=4) as ews_pool,
            tc.tile_pool(name="mp", bufs=3) as m_pool,
            tc.tile_pool(name="blp", bufs=3) as bl_pool,
            tc.tile_pool(name="ph2", bufs=2, space="PSUM") as ph2,
            tc.tile_pool(name="pm", bufs=4, space="PSUM") as pm,
            tc.tile_pool(name="pv", bufs=1, space="PSUM") as pv,
            tc.tile_pool(name="pd", bufs=1, space="PSUM") as pd,
        ):
            # critical-path data first: graph pair 0's inputs, then W1
            xc0 = xchunk_pool.tile([IN, 2 * NPG], XDT, tag="xc")
            nc.sync.dma_start(out=xc0[:], in_=xsT[:, 0:2 * NPG])
            W1_sb = consts.tile([IN, H], XDT)
            nc.sync.dma_start(out=W1_sb[:], in_=W1[:])
            adj0 = adj_pool.tile([128, 2 * KB * NPG], F8, tag="adj")
            nc.sync.dma_start(out=adj0[:], in_=adjT[0])
            ews0 = ews_pool.tile([128, 2 * KB * V], F16, tag="ews")
            nc.sync.dma_start(out=ews0[:], in_=ews[0])

            vW1_sb = consts.tile([H, H], F32)
            nc.scalar.dma_start(out=vW1_sb[:], in_=vW1[:])
            vW2_sb = consts.tile([H, H], F32)
            nc.scalar.dma_start(out=vW2_sb[:], in_=vW2s[:])
            mW1_sb = consts.tile([H, H], F32)
            nc.scalar.dma_start(out=mW1_sb[:], in_=mW1[:])
            mW2_sb = consts.tile([H, OUT], F32)
            nc.scalar.dma_start(out=mW2_sb[:], in_=mW2[:])
            vb1_sb = consts.tile([H, 1], F32)
            nc.scalar.dma_start(out=vb1_sb[:], in_=vb1[:])
            vb2_sb = consts.tile([H, 1], F32)
            nc.scalar.dma_start(out=vb2_sb[:], in_=vb2[:])
            mb1_sb = consts.tile([H, 1], F32)
            nc.scalar.dma_start(out=mb1_sb[:], in_=mb1[:])
            mb2_sb = consts.tile([OUT, 1], F32)
            nc.scalar.dma_start(out=mb2_sb[:], in_=mb2[:])
            if with_bias:
                biasR_sb = consts.tile([2, H], F16)
                nc.scalar.dma_start(out=biasR_sb[:], in_=biasR[:])

            virtT = consts.tile([H, GPC * V], F32)  # virt^T, all graphs side by side
            t1 = consts.tile([H, GPC * V], F32)
            t1s = consts.tile([H, GPC], F32)

            def emit_embed(j):
                # u = (dinv*x) @ W1, cast fp16, for graph pair j (1024 nodes)
                if j == 0:
                    xc = xc0
                else:
                    xc = xchunk_pool.tile([IN, 2 * NPG], XDT, tag="xc")
                    nc.sync.dma_start(out=xc[:], in_=xsT[:, 2 * j * NPG:2 * (j + 1) * NPG])
                u_j = u_pool.tile([128, 2 * KB * H], F16, tag="u")
                for half in range(2):
                    p_h2 = ph2.tile([128, KB * H], F32, tag="ph2")
                    for kb in range(KB):
                        nc.tensor.matmul(
                            p_h2[:, kb * H:(kb + 1) * H],
                            xc[:, half * NPG + kb * 128: half * NPG + (kb + 1) * 128],
                            W1_sb[:],
                            start=True, stop=True,
                        )
                    nc.vector.tensor_copy(
                        out=u_j[:, half * KB * H:(half + 1) * KB * H], in_=p_h2[:])
                return u_j

            us = [emit_embed(0)]
            pending = []
            for g in range(GPC):
                j, gg = divmod(g, 2)
                if gg == 0:
                    if j + 1 < GPC // 2:
                        us.append(emit_embed(j + 1))
                    if j == 0:
                        adj_pair, ews_pair = adj0, ews0
                    else:
                        adj_pair = adj_pool.tile([128, 2 * KB * NPG], F8, tag="adj")
                        nc.sync.dma_start(out=adj_pair[:], in_=adjT[j])
                        ews_pair = ews_pool.tile([128, 2 * KB * V], F16, tag="ews")
                        nc.sync.dma_start(out=ews_pair[:], in_=ews[j])
                u_g = us[j][:, gg * KB * H:(gg + 1) * KB * H]
                adj_sb = adj_pair[:, gg * KB * NPG:(gg + 1) * KB * NPG]
                ews_sb = ews_pair[:, gg * KB * V:(gg + 1) * KB * V]
                if with_bias:
                    bl_sb = bl_pool.tile([2, NPG], F16, tag="bl")
                    nc.sync.dma_start(out=bl_sb[:], in_=biasL[g])

                m_sb = m_pool.tile([128, KB * H], F16, tag="m")
                for mb in range(KB):
                    p_m = pm.tile([128, H], F32, tag="pm")
                    if with_bias:
                        nc.tensor.matmul(
                            p_m[:], bl_sb[:, mb * 128:(mb + 1) * 128], biasR_sb[:],
                            start=True, stop=False,
                        )
                    for kb in range(KB):
                        nc.tensor.matmul(
                            p_m[:],
                            adj_sb[:, kb * NPG + mb * 128: kb * NPG + (mb + 1) * 128],
                            u_g[:, kb * H:(kb + 1) * H],
                            start=(kb == 0 and not with_bias),
                            stop=(kb == KB - 1),
                        )
                    nc.scalar.activation(
                        out=m_sb[:, mb * H:(mb + 1) * H], in_=p_m[:],
                        func=mybir.ActivationFunctionType.Relu,
                    )

                # ---- pooling (deferred by one graph so the relu is long done
                # by the time the PE reaches these matmuls) ----
                pending.append((g, m_sb, ews_sb))
                emit_g = g - 1 if g < GPC - 1 else None
                ready = [p for p in pending if p[0] == emit_g]
                if g == GPC - 1:
                    ready = list(pending)
                for eg, e_m, e_ews in ready:
                    p_v = pv.tile([128, V], F32, tag="pv")
                    for kb in range(KB):
                        nc.tensor.matmul(
                            p_v[:],
                            e_m[:, kb * H:(kb + 1) * H],
                            e_ews[:, kb * V:(kb + 1) * V],
                            start=(kb == 0), stop=(kb == KB - 1),
                        )
                    nc.vector.tensor_copy(out=virtT[:, eg * V:(eg + 1) * V], in_=p_v[:])
                    pending.remove((eg, e_m, e_ews))

                # ---- MLP first stage per quarter once its 4 graphs are emitted ----
                for q in range(4):
                    if g != (4 * q + 5 if q < 3 else GPC - 1):
                        continue
                    p_t1 = pd.tile([128, 256], F32, tag="pd")
                    nc.tensor.matmul(
                        p_t1[:], vW1_sb[:], virtT[:, q * 256:(q + 1) * 256],
                        start=True, stop=True,
                    )
                    nc.scalar.activation(
                        out=t1[:, q * 256:(q + 1) * 256], in_=p_t1[:],
                        func=mybir.ActivationFunctionType.Relu, bias=vb1_sb[:],
                    )
                    nc.vector.tensor_reduce(
                        out=t1s[:, q * 4:(q + 1) * 4],
                        in_=t1[:, q * 256:(q + 1) * 256]
                            .rearrange("p (g v) -> p g v", v=V),
                        axis=mybir.AxisListType.X, op=mybir.AluOpType.add,
                    )

            # ---- rest of the MLP tail ----
            p_gf = pd.tile([128, GPC], F32, tag="pd")
            nc.tensor.matmul(p_gf[:], vW2_sb[:], t1s[:], start=True, stop=True)
            gf = consts.tile([H, GPC], F32)
            nc.scalar.activation(
                out=gf[:], in_=p_gf[:],
                func=mybir.ActivationFunctionType.Identity, bias=vb2_sb[:],
            )
            p_q1 = pd.tile([128, GPC], F32, tag="pd")
            nc.tensor.matmul(p_q1[:], mW1_sb[:], gf[:], start=True, stop=True)
            q1 = consts.tile([H, GPC], F32)
            nc.scalar.activation(
                out=q1[:], in_=p_q1[:],
                func=mybir.ActivationFunctionType.Relu, bias=mb1_sb[:],
            )
            p_o = pd.tile([OUT, GPC], F32, tag="pd")
            nc.tensor.matmul(p_o[:], mW2_sb[:], q1[:], start=True, stop=True)
            o_sb = consts.tile([OUT, GPC], F32)
            nc.scalar.activation(
                out=o_sb[:], in_=p_o[:],
                func=mybir.ActivationFunctionType.Identity, bias=mb2_sb[:],
            )
            nc.sync.dma_start(out=outT[:], in_=o_sb[:])

    nc.finalize()
    return nc


def _reference_numpy(x, edge_index, W_emb, b_emb, W_gcn, b_gcn, edge_weights,
                     vW1, vb1, vW2, vb2, mW1, mb1, mW2, mb2):
    """Pure-numpy fallback (used only if graphs are not disjoint)."""
    src, dst = edge_index[0].astype(np.int64), edge_index[1].astype(np.int64)
    h = x @ W_emb + b_emb
    h2 = h @ W_gcn
    deg = np.bincount(dst, minlength=N).astype(np.float32) + 1.0
    dinv = 1.0 / np.sqrt(deg)
    m = np.zeros_like(h2)
    np.add.at(m, dst, h2[src] * (dinv[src] * dinv[dst])[:, None])
    m += h2 * (dinv * dinv)[:, None]
    m = np.maximum(m + b_gcn, 0.0)
    hg = m.reshape(G, NPG, -1)
    virt = np.einsum('gnv,gnh->gvh', edge_weights, hg)
    t1 = np.maximum(virt @ vW1 + vb1, 0.0) @ vW2 + vb2
    gf = t1.mean(axis=1)
    return np.maximum(gf @ mW1 + mb1, 0.0) @ mW2 + mb2


def kernel(x, edge_index, batch, W_emb, b_emb, W_gcn, b_gcn, edge_weights,
           vW1, vb1, vW2, vb2, mW1, mb1, mW2, mb2, _trace=False):
    x = np.asarray(x, dtype=np.float32)
    edge_index = np.asarray(edge_index, dtype=np.int32)
    W_emb = np.asarray(W_emb, dtype=np.float32)
    b_emb = np.asarray(b_emb, dtype=np.float32)
    W_gcn = np.asarray(W_gcn, dtype=np.float32)
    b_gcn = np.asarray(b_gcn, dtype=np.float32)
    edge_weights = np.asarray(edge_weights, dtype=np.float32)
    vW1, vb1 = np.asarray(vW1, np.float32), np.asarray(vb1, np.float32)
    vW2, vb2 = np.asarray(vW2, np.float32), np.asarray(vb2, np.float32)
    mW1, mb1 = np.asarray(mW1, np.float32), np.asarray(mb1, np.float32)
    mW2, mb2 = np.asarray(mW2, np.float32), np.asarray(mb2, np.float32)

    src = edge_index[0].astype(np.int64)
    dst = edge_index[1].astype(np.int64)
    if not np.array_equal(src // NPG, dst // NPG):
        # cross-graph edges: dense per-graph adjacency doesn't apply
        return _reference_numpy(x, edge_index, W_emb, b_emb, W_gcn, b_gcn,
                                edge_weights, vW1, vb1, vW2, vb2, mW1, mb1,
                                mW2, mb2).astype(np.float32)

    # ---- host prep ----
    deg = (np.bincount(dst, minlength=N) + 1).astype(np.float32)  # in-degree + self loop
    dinv = (1.0 / np.sqrt(deg)).astype(np.float32)

    # per-graph transposed adjacency counts (+ self loops), exact small ints in fp8e4
    gidx = src // NPG
    lin = (gidx * NPG + (src % NPG)) * NPG + (dst % NPG)
    counts = np.bincount(lin, minlength=G * NPG * NPG)
    adjT_all = counts.reshape(G, NPG, NPG).astype(np.float32)
    diag = np.arange(NPG)
    adjT_all[:, diag, diag] += np.float32(1.0)
    if adjT_all.max() > 16:  # not exactly representable in fp8e4
        return _reference_numpy(x, edge_index, W_emb, b_emb, W_gcn, b_gcn,
                                edge_weights, vW1, vb1, vW2, vb2, mW1, mb1,
                                mW2, mb2).astype(np.float32)
    adjT_all = adjT_all.astype(ml_dtypes.float8_e4m3)
    # SBUF layout: [g, p, kb*NPG + d], then merge graph pairs so each DMA is
    # one [128, contiguous] block covering 2 graphs
    adjT_sb_all = (
        adjT_all.reshape(G, KB, 128, NPG).transpose(0, 2, 1, 3).reshape(G, 128, KB * NPG)
    )
    adjT_sb_all = np.ascontiguousarray(
        adjT_sb_all.reshape(G // 2, 2, 128, KB * NPG).transpose(0, 2, 1, 3)
        .reshape(G // 2, 128, 2 * KB * NPG)
    )

    bvec = (b_emb @ W_gcn).astype(np.float32)
    with_bias = bool(np.any(bvec) or np.any(b_gcn))
    ew_col = edge_weights[:, :, 0]
    uniform = bool(np.all(edge_weights == ew_col[:, :, None]))
    W1h = (W_emb @ W_gcn).astype(np.float16)

    if uniform and not with_bias:
        # ---- fast path ----
        xs = (x * dinv[:, None]).astype(np.float16)      # fold D^-1/2_src into x
        # pooling weights: edge_weights column * dinv_dst
        cd = (ew_col * dinv.reshape(G, NPG)).astype(np.float16)  # [G, NPG]

        wf32_np = np.zeros((128, _C_W32), np.float32)
        wf32_np[:, _C_VW1:_C_VW1 + H] = vW1
        wf32_np[:, _C_VW2:_C_VW2 + H] = vW2
        wf32_np[:, _C_MW1:_C_MW1 + H] = mW1
        wf32_np[:, _C_MW2:_C_MW2 + OUT] = mW2
        wf32_np[:, _C_VB1] = vb1
        wf32_np[:, _C_VB2] = vb2
        wf32_np[:, _C_MB1] = mb1
        wf32_np[:OUT, _C_MB2] = mb2

        if "fast" not in _CACHE:
            _CACHE["fast"] = _build_fast()
        nc = _CACHE["fast"]

        # quad-merge adjacency pairs: [G//4, 128, 2 * 2*KB*NPG]
        adjT_quads = np.ascontiguousarray(
            adjT_sb_all.reshape(G // 4, 2, 128, 2 * KB * NPG).transpose(0, 2, 1, 3)
            .reshape(G // 4, 128, 4 * KB * NPG)
        )

        in_maps = []
        for c in range(N_CORES):
            xs_c = xs[c * NS:(c + 1) * NS]  # [8192, 128]
            xp_np = np.ascontiguousarray(
                xs_c.reshape(GPC * KB, 128, IN).transpose(1, 0, 2)
                .reshape(128, GPC * KB * IN)
            )
            cd_c = cd[c * GPC:(c + 1) * GPC]  # [GPC, NPG]
            cdp = np.ascontiguousarray(
                cd_c.reshape(GPC, KB, 128).transpose(2, 0, 1).reshape(128, GPC * KB)
            )
            wf16_np = np.concatenate([W1h, cdp], axis=1)  # [128, IN + GPC*KB]
            qs = slice(c * GPC // 4, (c + 1) * GPC // 4)
            in_maps.append({
                "xp": xp_np,
                "adjT": adjT_quads[qs],
                "wf16": np.ascontiguousarray(wf16_np),
                "wf32": wf32_np,
            })
    else:
        # ---- general path ----
        xdt = np.float16 if X_FP16 else np.float32
        xs = (x * dinv[:, None])  # fold D^-1/2 into x rows
        xsT_np = np.ascontiguousarray(xs.T.astype(xdt))  # [IN, N]
        ews_all = (edge_weights * dinv.reshape(G, NPG, 1)).astype(np.float16)
        ews_sb_all = (
            ews_all.reshape(G, KB, 128, V).transpose(0, 2, 1, 3).reshape(G, 128, KB * V)
        )
        ews_sb_all = np.ascontiguousarray(
            ews_sb_all.reshape(G // 2, 2, 128, KB * V).transpose(0, 2, 1, 3)
            .reshape(G // 2, 128, 2 * KB * V)
        )

        vW2s_h = (vW2 / np.float32(V)).astype(np.float32)
        if with_bias:
            # m-psum bias = wvec ⊗ bvec + sqrt(deg) ⊗ b_gcn, with
            # wvec = (Adj+I) @ dinv per graph (host-computable rank-2 correction)
            dinv_g = dinv.reshape(G, NPG)
            wvec = np.einsum('gsd,gs->gd', adjT_all.astype(np.float32), dinv_g)
            sdeg = np.sqrt(deg).reshape(G, NPG)
            biasL_all = np.stack([wvec, sdeg], axis=1).astype(np.float16)  # [G, 2, NPG]
            biasR_np = np.stack([bvec, b_gcn], axis=0).astype(np.float16)  # [2, H]

        key = with_bias
        if key not in _CACHE:
            _CACHE[key] = _build_program(with_bias)
        nc = _CACHE[key]

        in_maps = []
        for c in range(N_CORES):
            gs = slice(c * GPC, (c + 1) * GPC)
            ps = slice(c * GPC // 2, (c + 1) * GPC // 2)
            im = {
                "xsT": np.ascontiguousarray(xsT_np[:, c * NS:(c + 1) * NS]),
                "W1": W1h if X_FP16 else (W_emb @ W_gcn).astype(np.float32),
                "adjT": adjT_sb_all[ps],
                "ews": ews_sb_all[ps],
                "vW1": vW1, "vb1": vb1.reshape(H, 1),
                "vW2s": vW2s_h, "vb2": vb2.reshape(H, 1),
                "mW1": mW1, "mb1": mb1.reshape(H, 1),
                "mW2": mW2, "mb2": mb2.reshape(OUT, 1),
            }
            if with_bias:
                im["biasL"] = np.ascontiguousarray(biasL_all[gs])
                im["biasR"] = biasR_np
            in_maps.append(im)

    res = run_bass_kernel_spmd(
        nc, in_maps, core_ids=list(range(N_CORES)), trace=_trace,
    )
    out = np.concatenate([res.results[c]["outT"].T for c in range(N_CORES)], axis=0)
    if _trace:
        kernel.last_exec_time_ns = res.exec_time_ns
        kernel.last_results = res
    return out.astype(np.float32)


# revision 7
# speedup vs baseline: 1.7063x; 1.0708x over previous
"""Trainium2 Bass kernel for BalancedIPRMPNN (GNN message passing).

Reference computation (G=128 disjoint graphs, NPG=512 nodes each, H=128):
    h2   = x @ (W_emb @ W_gcn) + b_emb @ W_gcn          # embedding+GCN linear folded
    m    = relu(D^-1/2 (Adj + I) D^-1/2 @ h2 + b_gcn)   # GCN propagate (per graph)
    virt = einsum('gnv,gnh->gvh', edge_weights, m)      # weighted pooling (V=64)
    t1   = relu(virt @ vW1 + vb1)
    gf   = mean_v(t1 @ vW2 + vb2)
    out  = relu(gf @ mW1 + mb1) @ mW2 + mb2             # [G, 10]

Strategy: data-parallel over graphs, 16 graphs per core on 8 cores.  Message
passing runs as a dense per-graph [512,512] adjacency matmul on the tensor
engine (integer counts exact in fp8e4); the symmetric degree normalization is
folded into x (rows pre-scaled by dinv_src) and the pooling weights
(pre-scaled by dinv_dst, legal since relu commutes with a positive scale when
b_gcn == 0).

Fast path (edge_weights uniform across the virtual-node axis, zero biases —
the shipped init): virt rows are v-independent, so the whole virtual-node
block collapses to a single weighted node-sum per graph.  Kernel chain per
graph g:
    BT_g  = sum_kb  xs_blk^T @ Adj_blk         (PE, adj is the fp8 moving op)
    m_g   = BT_g^T @ W1                        (PE, via SBUF-cast of BT)
    mr_g  = relu(m_g)                          (ScalarE, one [128,512] pass)
    virtT[:, g] = sum_mb mr_blk^T @ cdinv_blk  (PE, weighted pooling)
then a tiny fp32 MLP tail over virtT [H, 16].  All bulk DMAs are issued
up-front into persistent SBUF tiles on two queues so the wire never idles,
and a few dummy matmuls warm the PE clock gate during the initial DMA wait.

Non-uniform edge_weights or nonzero biases fall back to the general program
(same math as the reference, dense per-graph pooling); non-disjoint graphs or
adjacency counts not exactly representable in fp8 fall back to numpy.
"""

import ml_dtypes
import numpy as np

import concourse.mybir as mybir
import concourse.tile as tile
from concourse import bacc
from concourse.bass_utils import run_bass_kernel_spmd

# Problem constants (hardcoded per contract)
G, NPG, H, IN, OUT, V = 128, 512, 128, 128, 10, 64
N = G * NPG
N_CORES = 8
GPC = G // N_CORES          # graphs per core = 16
NS = GPC * NPG              # nodes per core  = 8192
KB = NPG // 128             # 4 k-blocks of 128 nodes per graph

F32 = mybir.dt.float32
F16 = mybir.dt.float16
F8 = mybir.dt.float8e4

X_FP16 = True               # ship x in fp16 (halves x DMA, 4x faster h2 matmuls)

_CACHE = {}

# fp16 const-pack column offsets (fast path): W1, cdinv, vW1, vW2, mW1, mW2
_C_W1, _C_CD = 0, 128
_C_VW1, _C_VW2, _C_MW1, _C_MW2 = 192, 320, 448, 576
_C_W16 = 586
# fp32 const-pack (biases): vb1, vb2, mb1, mb2
_C_VB1, _C_VB2, _C_MB1, _C_MB2 = 0, 1, 2, 3
_C_W32 = 4


def _build_fast():
    """Fast-path program: uniform edge_weights, zero biases."""
    nc = bacc.Bacc("TRN2", target_bir_lowering=False)

    PR = 2 * KB * NPG  # adjacency columns per graph pair (4096)

    xp = nc.dram_tensor("xp", [128, GPC * KB * IN], F16, kind="ExternalInput")
    # adjacency counts (+I), quad-merged: [q, p, jj*PR + gg*KB*NPG + kb*NPG + d]
    adjT = nc.dram_tensor("adjT", [GPC // 4, 128, 2 * PR], F8, kind="ExternalInput")
    # fp16 consts: W1 = W_emb @ W_gcn, cdinv pooling vecs, tail MLP weights
    wf16 = nc.dram_tensor("wf16", [128, _C_W16], F16, kind="ExternalInput")
    # fp32 consts: tail bias columns
    wf32 = nc.dram_tensor("wf32", [128, _C_W32], F32, kind="ExternalInput")
    outT = nc.dram_tensor("outT", [OUT, GPC], F32, kind="ExternalOutput")

    with tile.TileContext(nc) as tc:
        with (
            tc.tile_pool(name="consts", bufs=1) as consts,
            tc.tile_pool(name="btsb", bufs=3) as bt_pool,
            tc.tile_pool(name="mrel", bufs=3) as mr_pool,
            tc.tile_pool(name="pbt", bufs=2, space="PSUM") as pbt,
            tc.tile_pool(name="pm", bufs=2, space="PSUM") as pm,
            tc.tile_pool(name="pv", bufs=1, space="PSUM") as pv,
            tc.tile_pool(name="pwarm", bufs=1, space="PSUM") as pwarm,
            tc.tile_pool(name="ptail", bufs=2, space="PSUM") as ptail,
        ):
            # ---- bulk DMA plan -------------------------------------------
            # Everything rides the two HWDGE queues.  Emission order keeps
            # the 8 hardware DGE semaphores from being reused by a DMA whose
            # reuse-guard (wait for the prior use's consumers) could stall a
            # time-critical transfer: wf16 takes a never-reused sem; the
            # sync-queue bulk stream is ordered by consumption, and only its
            # tail entries (xp2, q67, xp3, wf32) land on reused sems, whose
            # guards (early graphs' matmuls) clear long before they matter.
            wf16_sb = consts.tile([128, _C_W16], F16)
            nc.scalar.dma_start(out=wf16_sb[:], in_=wf16[:])
            xp_sb = consts.tile([128, GPC * KB * IN], F16)
            adj_all = consts.tile([128, (GPC // 2) * PR], F8)
            wf32_sb = consts.tile([128, _C_W32], F32)

            XC = KB * IN * 4  # xp columns per 4-graph chunk (2048)

            def xp_dma(q):
                nc.sync.dma_start(out=xp_sb[:, q * XC:(q + 1) * XC],
                                  in_=xp[:, q * XC:(q + 1) * XC])

            def adj_dma(lo, hi):  # pairs [lo, hi) as one DMA (within one quad)
                assert lo // 2 == (hi - 1) // 2
                nc.sync.dma_start(
                    out=adj_all[:, lo * PR:hi * PR],
                    in_=adjT[lo // 2, :, (lo % 2) * PR:(lo % 2 + hi - lo) * PR],
                )

            adj_dma(0, 1)
            xp_dma(0)
            adj_dma(1, 2)
            adj_dma(2, 3)
            xp_dma(1)
            adj_dma(3, 4)
            adj_dma(4, 6)
            xp_dma(2)
            adj_dma(6, 8)
            xp_dma(3)
            nc.sync.dma_start(out=wf32_sb[:], in_=wf32[:])

            W1_sb = wf16_sb[:, _C_W1:_C_W1 + IN]
            cdinv = wf16_sb[:, _C_CD:_C_CD + GPC * KB]

            # ---- PE warm-up during the initial DMA wait ----
            scratch = consts.tile([128, NPG], F16)
            nc.gpsimd.memset(scratch[:], 0.0)
            p_w = pwarm.tile([128, NPG], F32, tag="warm")
            for _ in range(6):
                nc.tensor.matmul(p_w[:], scratch[:, 0:128], scratch[:],
                                 start=True, stop=True)

            p_v = pv.tile([128, GPC], F32, tag="pv")

            bts, mrs = {}, {}

            def st1(g):
                j, gg = divmod(g, 2)
                base = j * PR + gg * KB * NPG
                p_bt = pbt.tile([128, NPG], F32, tag="bt")
                for kb in range(KB):
                    nc.tensor.matmul(
                        p_bt[:],
                        xp_sb[:, (g * KB + kb) * IN:(g * KB + kb + 1) * IN],
                        adj_all[:, base + kb * NPG: base + (kb + 1) * NPG],
                        start=(kb == 0), stop=(kb == KB - 1),
                    )
                bt_sb = bt_pool.tile([128, NPG], F16, tag="btsb")
                nc.vector.tensor_copy(out=bt_sb[:], in_=p_bt[:])
                bts[g] = bt_sb

            def st2(g):
                p_m = pm.tile([128, KB * H], F32, tag="m")
                for mb in range(KB):
                    nc.tensor.matmul(
                        p_m[:, mb * H:(mb + 1) * H],
                        bts[g][:, mb * 128:(mb + 1) * 128],
                        W1_sb,
                        start=True, stop=True,
                    )
                mr = mr_pool.tile([128, KB * H], F16, tag="mr")
                nc.scalar.activation(
                    out=mr[:], in_=p_m[:],
                    func=mybir.ActivationFunctionType.Relu,
                )
                mrs[g] = mr
                del bts[g]

            def pool(g):
                for mb in range(KB):
                    nc.tensor.matmul(
                        p_v[:, g:g + 1],
                        mrs[g][:, mb * H:(mb + 1) * H],
                        cdinv[:, g * KB + mb:g * KB + mb + 1],
                        start=(mb == 0), stop=(mb == KB - 1),
                    )
                del mrs[g]

            for it in range(GPC + 3):
                if it < GPC:
                    st1(it)
                if 2 <= it < GPC + 2:
                    st2(it - 2)
                if it >= 3:
                    pool(it - 3)

            # ---- fp16 MLP tail over virtT [H, GPC] ----
            virtT = consts.tile([H, GPC], F16, tag="virtT")
            nc.vector.tensor_copy(out=virtT[:], in_=p_v[:])
            p_t1 = ptail.tile([128, GPC], F32, tag="tail")
            nc.tensor.matmul(p_t1[:], wf16_sb[:, _C_VW1:_C_VW1 + H], virtT[:],
                             start=True, stop=True)
            t1 = consts.tile([H, GPC], F16, tag="t1")
            nc.scalar.activation(
                out=t1[:], in_=p_t1[:],
                func=mybir.ActivationFunctionType.Relu,
                bias=wf32_sb[:, _C_VB1:_C_VB1 + 1],
            )
            p_gf = ptail.tile([128, GPC], F32, tag="tail")
            nc.tensor.matmul(p_gf[:], wf16_sb[:, _C_VW2:_C_VW2 + H], t1[:],
                             start=True, stop=True)
            gf = consts.tile([H, GPC], F16, tag="gf")
            nc.scalar.activation(
                out=gf[:], in_=p_gf[:],
                func=mybir.ActivationFunctionType.Identity,
                bias=wf32_sb[:, _C_VB2:_C_VB2 + 1],
            )
            p_q1 = ptail.tile([128, GPC], F32, tag="tail")
            nc.tensor.matmul(p_q1[:], wf16_sb[:, _C_MW1:_C_MW1 + H], gf[:],
                             start=True, stop=True)
            q1 = consts.tile([H, GPC], F16, tag="q1")
            nc.scalar.activation(
                out=q1[:], in_=p_q1[:],
                func=mybir.ActivationFunctionType.Relu,
                bias=wf32_sb[:, _C_MB1:_C_MB1 + 1],
            )
            p_o = ptail.tile([OUT, GPC], F32, tag="tail")
            nc.tensor.matmul(p_o[:], wf16_sb[:, _C_MW2:_C_MW2 + OUT], q1[:],
                             start=True, stop=True)
            o_sb = consts.tile([OUT, GPC], F32, tag="osb")
            nc.scalar.activation(
                out=o_sb[:], in_=p_o[:],
                func=mybir.ActivationFunctionType.Identity,
                bias=wf32_sb[0:OUT, _C_MB2:_C_MB2 + 1],
            )
            nc.scalar.dma_start(out=outT[:], in_=o_sb[:])

    nc.finalize()
    return nc


def _build_program(with_bias: bool):
    """General per-core program (any edge_weights / biases)."""
    nc = bacc.Bacc("TRN2", target_bir_lowering=False)
    XDT = F16 if X_FP16 else F32

    # ---- DRAM I/O ----
    xsT = nc.dram_tensor("xsT", [IN, NS], XDT, kind="ExternalInput")          # dinv-scaled x, transposed
    W1 = nc.dram_tensor("W1", [IN, H], XDT, kind="ExternalInput")             # W_emb @ W_gcn
    # adjacency counts (+I), pre-arranged to SBUF layout, 2 graphs per row:
    # [j, p, gg*KB*NPG + kb*NPG + d]  (gg in {0,1}, graph = 2j+gg)
    adjT = nc.dram_tensor("adjT", [GPC // 2, 128, 2 * KB * NPG], F8, kind="ExternalInput")
    # dinv-scaled edge_weights, pre-arranged likewise
    ews = nc.dram_tensor("ews", [GPC // 2, 128, 2 * KB * V], F16, kind="ExternalInput")
    vW1 = nc.dram_tensor("vW1", [H, H], F32, kind="ExternalInput")
    vb1 = nc.dram_tensor("vb1", [H, 1], F32, kind="ExternalInput")
    vW2s = nc.dram_tensor("vW2s", [H, H], F32, kind="ExternalInput")          # vW2 / V
    vb2 = nc.dram_tensor("vb2", [H, 1], F32, kind="ExternalInput")
    mW1 = nc.dram_tensor("mW1", [H, H], F32, kind="ExternalInput")
    mb1 = nc.dram_tensor("mb1", [H, 1], F32, kind="ExternalInput")
    mW2 = nc.dram_tensor("mW2", [H, OUT], F32, kind="ExternalInput")
    mb2 = nc.dram_tensor("mb2", [OUT, 1], F32, kind="ExternalInput")
    if with_bias:
        biasL = nc.dram_tensor("biasL", [GPC, 2, NPG], F16, kind="ExternalInput")
        biasR = nc.dram_tensor("biasR", [2, H], F16, kind="ExternalInput")
    outT = nc.dram_tensor("outT", [OUT, GPC], F32, kind="ExternalOutput")

    with tile.TileContext(nc) as tc:
        with (
            tc.tile_pool(name="consts", bufs=1) as consts,
            tc.tile_pool(name="xchunk", bufs=4) as xchunk_pool,
            tc.tile_pool(name="upool", bufs=3) as u_pool,
            tc.tile_pool(name="adj", bufs=4) as adj_pool,
            tc.tile_pool(name="ewsp", bufs=4) as ews_pool,
            tc.tile_pool(name="mp", bufs=3) as m_pool,
            tc.tile_pool(name="blp", bufs=3) as bl_pool,
            tc.tile_pool(name="ph2", bufs=2, space="PSUM") as ph2,
            tc.tile_pool(name="pm", bufs=4, space="PSUM") as pm,
            tc.tile_pool(name="pv", bufs=1, space="PSUM") as pv,
            tc.tile_pool(name="pd", bufs=1, space="PSUM") as pd,
        ):
            # critical-path data first: graph pair 0's inputs, then W1
            xc0 = xchunk_pool.tile([IN, 2 * NPG], XDT, tag="xc")
            nc.sync.dma_start(out=xc0[:], in_=xsT[:, 0:2 * NPG])
            W1_sb = consts.tile([IN, H], XDT)
            nc.sync.dma_start(out=W1_sb[:], in_=W1[:])
            adj0 = adj_pool.tile([128, 2 * KB * NPG], F8, tag="adj")
            nc.sync.dma_start(out=adj0[:], in_=adjT[0])
            ews0 = ews_pool.tile([128, 2 * KB * V], F16, tag="ews")
            nc.sync.dma_start(out=ews0[:], in_=ews[0])

            vW1_sb = consts.tile([H, H], F32)
            nc.scalar.dma_start(out=vW1_sb[:], in_=vW1[:])
            vW2_sb = consts.tile([H, H], F32)
            nc.scalar.dma_start(out=vW2_sb[:], in_=vW2s[:])
            mW1_sb = consts.tile([H, H], F32)
            nc.scalar.dma_start(out=mW1_sb[:], in_=mW1[:])
            mW2_sb = consts.tile([H, OUT], F32)
            nc.scalar.dma_start(out=mW2_sb[:], in_=mW2[:])
            vb1_sb = consts.tile([H, 1], F32)
            nc.scalar.dma_start(out=vb1_sb[:], in_=vb1[:])
            vb2_sb = consts.tile([H, 1], F32)
            nc.scalar.dma_start(out=vb2_sb[:], in_=vb2[:])
            mb1_sb = consts.tile([H, 1], F32)
            nc.scalar.dma_start(out=mb1_sb[:], in_=mb1[:])
            mb2_sb = consts.tile([OUT, 1], F32)
            nc.scalar.dma_start(out=mb2_sb[:], in_=mb2[:])
            if with_bias:
                biasR_sb = consts.tile([2, H], F16)
                nc.scalar.dma_start(out=biasR_sb[:], in_=biasR[:])

            virtT = consts.tile([H, GPC * V], F32)  # virt^T, all graphs side by side
            t1 = consts.tile([H, GPC * V], F32)
            t1s = consts.tile([H, GPC], F32)

            def emit_embed(j):
                # u = (dinv*x) @ W1, cast fp16, for graph pair j (1024 nodes)
                if j == 0:
                    xc = xc0
                else:
                    xc = xchunk_pool.tile([IN, 2 * NPG], XDT, tag="xc")
                    nc.sync.dma_start(out=xc[:], in_=xsT[:, 2 * j * NPG:2 * (j + 1) * NPG])
                u_j = u_pool.tile([128, 2 * KB * H], F16, tag="u")
                for half in range(2):
                    p_h2 = ph2.tile([128, KB * H], F32, tag="ph2")
                    for kb in range(KB):
                        nc.tensor.matmul(
                            p_h2[:, kb * H:(kb + 1) * H],
                            xc[:, half * NPG + kb * 128: half * NPG + (kb + 1) * 128],
                            W1_sb[:],
                            start=True, stop=True,
                        )
                    nc.vector.tensor_copy(
                        out=u_j[:, half * KB * H:(half + 1) * KB * H], in_=p_h2[:])
                return u_j

            us = [emit_embed(0)]
            pending = []
            for g in range(GPC):
                j, gg = divmod(g, 2)
                if gg == 0:
                    if j + 1 < GPC // 2:
                        us.append(emit_embed(j + 1))
                    if j == 0:
                        adj_pair, ews_pair = adj0, ews0
                    else:
                        adj_pair = adj_pool.tile([128, 2 * KB * NPG], F8, tag="adj")
                        nc.sync.dma_start(out=adj_pair[:], in_=adjT[j])
                        ews_pair = ews_pool.tile([128, 2 * KB * V], F16, tag="ews")
                        nc.sync.dma_start(out=ews_pair[:], in_=ews[j])
                u_g = us[j][:, gg * KB * H:(gg + 1) * KB * H]
                adj_sb = adj_pair[:, gg * KB * NPG:(gg + 1) * KB * NPG]
                ews_sb = ews_pair[:, gg * KB * V:(gg + 1) * KB * V]
                if with_bias:
                    bl_sb = bl_pool.tile([2, NPG], F16, tag="bl")
                    nc.sync.dma_start(out=bl_sb[:], in_=biasL[g])

                m_sb = m_pool.tile([128, KB * H], F16, tag="m")
                for mb in range(KB):
                    p_m = pm.tile([128, H], F32, tag="pm")
                    if with_bias:
                        nc.tensor.matmul(
                            p_m[:], bl_sb[:, mb * 128:(mb + 1) * 128], biasR_sb[:],
                            start=True, stop=False,
                        )
                    for kb in range(KB):
                        nc.tensor.matmul(
                            p_m[:],
                            adj_sb[:, kb * NPG + mb * 128: kb * NPG + (mb + 1) * 128],
                            u_g[:, kb * H:(kb + 1) * H],
                            start=(kb == 0 and not with_bias),
                            stop=(kb == KB - 1),
                        )
                    nc.scalar.activation(
                        out=m_sb[:, mb * H:(mb + 1) * H], in_=p_m[:],
                        func=mybir.ActivationFunctionType.Relu,
                    )

                # ---- pooling (deferred by one graph so the relu is long done
                # by the time the PE reaches these matmuls) ----
                pending.append((g, m_sb, ews_sb))
                emit_g = g - 1 if g < GPC - 1 else None
                ready = [p for p in pending if p[0] == emit_g]
                if g == GPC - 1:
                    ready = list(pending)
                for eg, e_m, e_ews in ready:
                    p_v = pv.tile([128, V], F32, tag="pv")
                    for kb in range(KB):
                        nc.tensor.matmul(
                            p_v[:],
                            e_m[:, kb * H:(kb + 1) * H],
                            e_ews[:, kb * V:(kb + 1) * V],
                            start=(kb == 0), stop=(kb == KB - 1),
                        )
                    nc.vector.tensor_copy(out=virtT[:, eg * V:(eg + 1) * V], in_=p_v[:])
                    pending.remove((eg, e_m, e_ews))

                # ---- MLP first stage per quarter once its 4 graphs are emitted ----
                for q in range(4):
                    if g != (4 * q + 5 if q < 3 else GPC - 1):
                        continue
                    p_t1 = pd.tile([128, 256], F32, tag="pd")
                    nc.tensor.matmul(
                        p_t1[:], vW1_sb[:], virtT[:, q * 256:(q + 1) * 256],
                        start=True, stop=True,
                    )
                    nc.scalar.activation(
                        out=t1[:, q * 256:(q + 1) * 256], in_=p_t1[:],
                        func=mybir.ActivationFunctionType.Relu, bias=vb1_sb[:],
                    )
                    nc.vector.tensor_reduce(
                        out=t1s[:, q * 4:(q + 1) * 4],
                        in_=t1[:, q * 256:(q + 1) * 256]
                            .rearrange("p (g v) -> p g v", v=V),
                        axis=mybir.AxisListType.X, op=mybir.AluOpType.add,
                    )

            # ---- rest of the MLP tail ----
            p_gf = pd.tile([128, GPC], F32, tag="pd")
            nc.tensor.matmul(p_gf[:], vW2_sb[:], t1s[:], start=True, stop=True)
            gf = consts.tile([H, GPC], F32)
            nc.scalar.activation(
                out=gf[:], in_=p_gf[:],
                func=mybir.ActivationFunctionType.Identity, bias=vb2_sb[:],
            )
            p_q1 = pd.tile([128, GPC], F32, tag="pd")
            nc.tensor.matmul(p_q1[:], mW1_sb[:], gf[:], start=True, stop=True)
            q1 = consts.tile([H, GPC], F32)
            nc.scalar.activation(
                out=q1[:], in_=p_q1[:],
                func=mybir.ActivationFunctionType.Relu, bias=mb1_sb[:],
            )
            p_o = pd.tile([OUT, GPC], F32, tag="pd")
            nc.tensor.matmul(p_o[:], mW2_sb[:], q1[:], start=True, stop=True)
            o_sb = consts.tile([OUT, GPC], F32)
            nc.scalar.activation(
                out=o_sb[:], in_=p_o[:],
                func=mybir.ActivationFunctionType.Identity, bias=mb2_sb[:],
            )
            nc.sync.dma_start(out=outT[:], in_=o_sb[:])

    nc.finalize()
    return nc


def _reference_numpy(x, edge_index, W_emb, b_emb, W_gcn, b_gcn, edge_weights,
                     vW1, vb1, vW2, vb2, mW1, mb1, mW2, mb2):
    """Pure-numpy fallback (used only if graphs are not disjoint)."""
    src, dst = edge_index[0].astype(np.int64), edge_index[1].astype(np.int64)
    h = x @ W_emb + b_emb
    h2 = h @ W_gcn
    deg = np.bincount(dst, minlength=N).astype(np.float32) + 1.0
    dinv = 1.0 / np.sqrt(deg)
    m = np.zeros_like(h2)
    np.add.at(m, dst, h2[src] * (dinv[src] * dinv[dst])[:, None])
    m += h2 * (dinv * dinv)[:, None]
    m = np.maximum(m + b_gcn, 0.0)
    hg = m.reshape(G, NPG, -1)
    virt = np.einsum('gnv,gnh->gvh', edge_weights, hg)
    t1 = np.maximum(virt @ vW1 + vb1, 0.0) @ vW2 + vb2
    gf = t1.mean(axis=1)
    return np.maximum(gf @ mW1 + mb1, 0.0) @ mW2 + mb2


def kernel(x, edge_index, batch, W_emb, b_emb, W_gcn, b_gcn, edge_weights,
           vW1, vb1, vW2, vb2, mW1, mb1, mW2, mb2, _trace=False):
    x = np.asarray(x, dtype=np.float32)
    edge_index = np.asarray(edge_index, dtype=np.int32)
    W_emb = np.asarray(W_emb, dtype=np.float32)
    b_emb = np.asarray(b_emb, dtype=np.float32)
    W_gcn = np.asarray(W_gcn, dtype=np.float32)
    b_gcn = np.asarray(b_gcn, dtype=np.float32)
    edge_weights = np.asarray(edge_weights, dtype=np.float32)
    vW1, vb1 = np.asarray(vW1, np.float32), np.asarray(vb1, np.float32)
    vW2, vb2 = np.asarray(vW2, np.float32), np.asarray(vb2, np.float32)
    mW1, mb1 = np.asarray(mW1, np.float32), np.asarray(mb1, np.float32)
    mW2, mb2 = np.asarray(mW2, np.float32), np.asarray(mb2, np.float32)

    src = edge_index[0].astype(np.int64)
    dst = edge_index[1].astype(np.int64)
    if not np.array_equal(src // NPG, dst // NPG):
        # cross-graph edges: dense per-graph adjacency doesn't apply
        return _reference_numpy(x, edge_index, W_emb, b_emb, W_gcn, b_gcn,
                                edge_weights, vW1, vb1, vW2, vb2, mW1, mb1,
                                mW2, mb2).astype(np.float32)

    # ---- host prep ----
    deg = (np.bincount(dst, minlength=N) + 1).astype(np.float32)  # in-degree + self loop
    dinv = (1.0 / np.sqrt(deg)).astype(np.float32)

    # per-graph transposed adjacency counts (+ self loops), exact small ints in fp8e4
    gidx = src // NPG
    lin = (gidx * NPG + (src % NPG)) * NPG + (dst % NPG)
    counts = np.bincount(lin, minlength=G * NPG * NPG)
    adjT_all = counts.reshape(G, NPG, NPG).astype(np.float32)
    diag = np.arange(NPG)
    adjT_all[:, diag, diag] += np.float32(1.0)
    if adjT_all.max() > 16:  # not exactly representable in fp8e4
        return _reference_numpy(x, edge_index, W_emb, b_emb, W_gcn, b_gcn,
                                edge_weights, vW1, vb1, vW2, vb2, mW1, mb1,
                                mW2, mb2).astype(np.float32)
    adjT_all = adjT_all.astype(ml_dtypes.float8_e4m3)
    # SBUF layout: [g, p, kb*NPG + d], then merge graph pairs so each DMA is
    # one [128, contiguous] block covering 2 graphs
    adjT_sb_all = (
        adjT_all.reshape(G, KB, 128, NPG).transpose(0, 2, 1, 3).reshape(G, 128, KB * NPG)
    )
    adjT_sb_all = np.ascontiguousarray(
        adjT_sb_all.reshape(G // 2, 2, 128, KB * NPG).transpose(0, 2, 1, 3)
        .reshape(G // 2, 128, 2 * KB * NPG)
    )

    bvec = (b_emb @ W_gcn).astype(np.float32)
    with_bias = bool(np.any(bvec) or np.any(b_gcn))
    ew_col = edge_weights[:, :, 0]
    uniform = bool(np.all(edge_weights == ew_col[:, :, None]))
    W1h = (W_emb @ W_gcn).astype(np.float16)

    if uniform and not with_bias:
        # ---- fast path ----
        xs = (x * dinv[:, None]).astype(np.float16)      # fold D^-1/2_src into x
        # pooling weights: edge_weights column * dinv_dst
        cd = (ew_col * dinv.reshape(G, NPG)).astype(np.float16)  # [G, NPG]

        wf32_np = np.zeros((128, _C_W32), np.float32)
        wf32_np[:, _C_VB1] = vb1
        wf32_np[:, _C_VB2] = vb2
        wf32_np[:, _C_MB1] = mb1
        wf32_np[:OUT, _C_MB2] = mb2

        if "fast" not in _CACHE:
            _CACHE["fast"] = _build_fast()
        nc = _CACHE["fast"]

        # quad-merge adjacency pairs: [G//4, 128, 2 * 2*KB*NPG]
        adjT_quads = np.ascontiguousarray(
            adjT_sb_all.reshape(G // 4, 2, 128, 2 * KB * NPG).transpose(0, 2, 1, 3)
            .reshape(G // 4, 128, 4 * KB * NPG)
        )

        in_maps = []
        for c in range(N_CORES):
            xs_c = xs[c * NS:(c + 1) * NS]  # [8192, 128]
            xp_np = np.ascontiguousarray(
                xs_c.reshape(GPC * KB, 128, IN).transpose(1, 0, 2)
                .reshape(128, GPC * KB * IN)
            )
            cd_c = cd[c * GPC:(c + 1) * GPC]  # [GPC, NPG]
            cdp = np.ascontiguousarray(
                cd_c.reshape(GPC, KB, 128).transpose(2, 0, 1).reshape(128, GPC * KB)
            )
            wf16_np = np.zeros((128, _C_W16), np.float16)
            wf16_np[:, _C_W1:_C_W1 + IN] = W1h
            wf16_np[:, _C_CD:_C_CD + GPC * KB] = cdp
            wf16_np[:, _C_VW1:_C_VW1 + H] = vW1.astype(np.float16)
            wf16_np[:, _C_VW2:_C_VW2 + H] = vW2.astype(np.float16)
            wf16_np[:, _C_MW1:_C_MW1 + H] = mW1.astype(np.float16)
            wf16_np[:, _C_MW2:_C_MW2 + OUT] = mW2.astype(np.float16)
            qs = slice(c * GPC // 4, (c + 1) * GPC // 4)
            in_maps.append({
                "xp": xp_np,
                "adjT": adjT_quads[qs],
                "wf16": np.ascontiguousarray(wf16_np),
                "wf32": wf32_np,
            })
    else:
        # ---- general path ----
        xdt = np.float16 if X_FP16 else np.float32
        xs = (x * dinv[:, None])  # fold D^-1/2 into x rows
        xsT_np = np.ascontiguousarray(xs.T.astype(xdt))  # [IN, N]
        ews_all = (edge_weights * dinv.reshape(G, NPG, 1)).astype(np.float16)
        ews_sb_all = (
            ews_all.reshape(G, KB, 128, V).transpose(0, 2, 1, 3).reshape(G, 128, KB * V)
        )
        ews_sb_all = np.ascontiguousarray(
            ews_sb_all.reshape(G // 2, 2, 128, KB * V).transpose(0, 2, 1, 3)
            .reshape(G // 2, 128, 2 * KB * V)
        )

        vW2s_h = (vW2 / np.float32(V)).astype(np.float32)
        if with_bias:
            # m-psum bias = wvec ⊗ bvec + sqrt(deg) ⊗ b_gcn, with
            # wvec = (Adj+I) @ dinv per graph (host-computable rank-2 correction)
            dinv_g = dinv.reshape(G, NPG)
            wvec = np.einsum('gsd,gs->gd', adjT_all.astype(np.float32), dinv_g)
            sdeg = np.sqrt(deg).reshape(G, NPG)
            biasL_all = np.stack([wvec, sdeg], axis=1).astype(np.float16)  # [G, 2, NPG]
            biasR_np = np.stack([bvec, b_gcn], axis=0).astype(np.float16)  # [2, H]

        key = with_bias
        if key not in _CACHE:
            _CACHE[key] = _build_program(with_bias)
        nc = _CACHE[key]

        in_maps = []
        for c in range(N_CORES):
            gs = slice(c * GPC, (c + 1) * GPC)
            ps = slice(c * GPC // 2, (c + 1) * GPC // 2)
            im = {
                "xsT": np.ascontiguousarray(xsT_np[:, c * NS:(c + 1) * NS]),
                "W1": W1h if X_FP16 else (W_emb @ W_gcn).astype(np.float32),
                "adjT": adjT_sb_all[ps],
                "ews": ews_sb_all[ps],
                "vW1": vW1, "vb1": vb1.reshape(H, 1),
                "vW2s": vW2s_h, "vb2": vb2.reshape(H, 1),
                "mW1": mW1, "mb1": mb1.reshape(H, 1),
                "mW2": mW2, "mb2": mb2.reshape(OUT, 1),
            }
            if with_bias:
                im["biasL"] = np.ascontiguousarray(biasL_all[gs])
                im["biasR"] = biasR_np
            in_maps.append(im)

    res = run_bass_kernel_spmd(
        nc, in_maps, core_ids=list(range(N_CORES)), trace=_trace,
    )
    out = np.concatenate([res.results[c]["outT"].T for c in range(N_CORES)], axis=0)
    if _trace:
        kernel.last_exec_time_ns = res.exec_time_ns
        kernel.last_results = res
    return out.astype(np.float32)


# revision 9
# speedup vs baseline: 1.7347x; 1.0166x over previous
"""Trainium2 Bass kernel for BalancedIPRMPNN (GNN message passing).

Reference computation (G=128 disjoint graphs, NPG=512 nodes each, H=128):
    h2   = x @ (W_emb @ W_gcn) + b_emb @ W_gcn          # embedding+GCN linear folded
    m    = relu(D^-1/2 (Adj + I) D^-1/2 @ h2 + b_gcn)   # GCN propagate (per graph)
    virt = einsum('gnv,gnh->gvh', edge_weights, m)      # weighted pooling (V=64)
    t1   = relu(virt @ vW1 + vb1)
    gf   = mean_v(t1 @ vW2 + vb2)
    out  = relu(gf @ mW1 + mb1) @ mW2 + mb2             # [G, 10]

Strategy: data-parallel over graphs, 16 graphs per core on 8 cores.  Message
passing runs as a dense per-graph [512,512] adjacency matmul on the tensor
engine (integer counts exact in fp8e4); the symmetric degree normalization is
folded into x (rows pre-scaled by dinv_src) and the pooling weights
(pre-scaled by dinv_dst, legal since relu commutes with a positive scale when
b_gcn == 0).

Fast path (edge_weights uniform across the virtual-node axis, zero biases —
the shipped init): virt rows are v-independent, so the whole virtual-node
block collapses to a single weighted node-sum per graph.  Kernel chain per
graph g:
    BT_g  = sum_kb  xs_blk^T @ Adj_blk         (PE, adj is the fp8 moving op)
    m_g   = BT_g^T @ W1                        (PE, via SBUF-cast of BT)
    mr_g  = relu(m_g)                          (ScalarE, one [128,512] pass)
    virtT[:, g] = sum_mb mr_blk^T @ cdinv_blk  (PE, weighted pooling)
then a tiny fp32 MLP tail over virtT [H, 16].  All bulk DMAs are issued
up-front into persistent SBUF tiles on two queues so the wire never idles,
and a few dummy matmuls warm the PE clock gate during the initial DMA wait.

Non-uniform edge_weights or nonzero biases fall back to the general program
(same math as the reference, dense per-graph pooling); non-disjoint graphs or
adjacency counts not exactly representable in fp8 fall back to numpy.
"""

import ml_dtypes
import numpy as np

import concourse.mybir as mybir
import concourse.tile as tile
from concourse import bacc
from concourse.bass_utils import run_bass_kernel_spmd

# Problem constants (hardcoded per contract)
G, NPG, H, IN, OUT, V = 128, 512, 128, 128, 10, 64
N = G * NPG
N_CORES = 8
GPC = G // N_CORES          # graphs per core = 16
NS = GPC * NPG              # nodes per core  = 8192
KB = NPG // 128             # 4 k-blocks of 128 nodes per graph

F32 = mybir.dt.float32
F16 = mybir.dt.float16
F8 = mybir.dt.float8e4

X_FP16 = True               # ship x in fp16 (halves x DMA, 4x faster h2 matmuls)

_CACHE = {}

# fp16 const-pack column offsets (fast path): W1, cdinv, vW1, vW2, mW1, mW2
_C_W1, _C_CD = 0, 128
_C_VW1, _C_VW2, _C_MW1, _C_MW2 = 192, 320, 448, 576
_C_W16 = 586
# fp32 const-pack (biases): vb1, vb2, mb1, mb2
_C_VB1, _C_VB2, _C_MB1, _C_MB2 = 0, 1, 2, 3
_C_W32 = 4


def _build_fast():
    """Fast-path program: uniform edge_weights, zero biases."""
    nc = bacc.Bacc("TRN2", target_bir_lowering=False)

    PR = 2 * KB * NPG  # adjacency columns per graph pair (4096)

    xp = nc.dram_tensor("xp", [128, GPC * KB * IN], F16, kind="ExternalInput")
    # adjacency counts (+I), quad-merged: [q, p, jj*PR + gg*KB*NPG + kb*NPG + d]
    adjT = nc.dram_tensor("adjT", [GPC // 4, 128, 2 * PR], F8, kind="ExternalInput")
    # fp16 consts: W1 = W_emb @ W_gcn, cdinv pooling vecs, tail MLP weights
    wf16 = nc.dram_tensor("wf16", [128, _C_W16], F16, kind="ExternalInput")
    # fp32 consts: tail bias columns
    wf32 = nc.dram_tensor("wf32", [128, _C_W32], F32, kind="ExternalInput")
    outT = nc.dram_tensor("outT", [OUT, GPC], F32, kind="ExternalOutput")

    with tile.TileContext(nc) as tc:
        with (
            tc.tile_pool(name="consts", bufs=1) as consts,
            tc.tile_pool(name="btsb", bufs=3) as bt_pool,
            tc.tile_pool(name="mrel", bufs=3) as mr_pool,
            tc.tile_pool(name="pbt", bufs=2, space="PSUM") as pbt,
            tc.tile_pool(name="pm", bufs=2, space="PSUM") as pm,
            tc.tile_pool(name="pv", bufs=1, space="PSUM") as pv,
            tc.tile_pool(name="pwarm", bufs=1, space="PSUM") as pwarm,
            tc.tile_pool(name="ptail", bufs=2, space="PSUM") as ptail,
        ):
            # ---- bulk DMA plan -------------------------------------------
            # Everything rides the two HWDGE queues.  Emission order keeps
            # the 8 hardware DGE semaphores from being reused by a DMA whose
            # reuse-guard (wait for the prior use's consumers) could stall a
            # time-critical transfer: wf16 takes a never-reused sem; the
            # sync-queue bulk stream is ordered by consumption, and only its
            # tail entries (xp2, q67, xp3, wf32) land on reused sems, whose
            # guards (early graphs' matmuls) clear long before they matter.
            wf16_sb = consts.tile([128, _C_W16], F16)
            nc.scalar.dma_start(out=wf16_sb[:], in_=wf16[:])
            xp_sb = consts.tile([128, GPC * KB * IN], F16)
            adj_all = consts.tile([128, (GPC // 2) * PR], F8)
            wf32_sb = consts.tile([128, _C_W32], F32)

            XC = KB * IN * 4  # xp columns per 4-graph chunk (2048)

            def xp_dma(q):
                nc.sync.dma_start(out=xp_sb[:, q * XC:(q + 1) * XC],
                                  in_=xp[:, q * XC:(q + 1) * XC])

            def adj_dma(lo, hi):  # pairs [lo, hi) as one DMA (within one quad)
                assert lo // 2 == (hi - 1) // 2
                nc.sync.dma_start(
                    out=adj_all[:, lo * PR:hi * PR],
                    in_=adjT[lo // 2, :, (lo % 2) * PR:(lo % 2 + hi - lo) * PR],
                )

            GR = KB * NPG   # adjacency columns per graph (2048)
            GX = KB * IN    # xp columns per graph (512)

            def adj_dma_g(g):  # one graph's adjacency
                nc.sync.dma_start(
                    out=adj_all[:, g * GR:(g + 1) * GR],
                    in_=adjT[g // 4, :, (g % 4) * GR:(g % 4 + 1) * GR],
                )

            def xp_dma_g(lo, hi):  # graphs [lo, hi) of xp
                nc.sync.dma_start(out=xp_sb[:, lo * GX:hi * GX],
                                  in_=xp[:, lo * GX:hi * GX])

            # tiny first chunks so graph 0 can start ASAP, then big strides
            adj_dma_g(0)
            xp_dma_g(0, 2)
            adj_dma_g(1)
            xp_dma_g(2, 4)
            adj_dma(1, 2)
            adj_dma(2, 3)
            xp_dma(1)
            adj_dma(3, 4)
            adj_dma(4, 6)
            xp_dma(2)
            adj_dma(6, 8)
            xp_dma(3)
            nc.sync.dma_start(out=wf32_sb[:], in_=wf32[:])

            W1_sb = wf16_sb[:, _C_W1:_C_W1 + IN]
            cdinv = wf16_sb[:, _C_CD:_C_CD + GPC * KB]

            # ---- PE warm-up during the initial DMA wait ----
            scratch = consts.tile([128, NPG], F16)
            nc.gpsimd.memset(scratch[:], 0.0)
            p_w = pwarm.tile([128, NPG], F32, tag="warm")
            for _ in range(8):
                nc.tensor.matmul(p_w[:], scratch[:, 0:128], scratch[:],
                                 start=True, stop=True)

            p_v = pv.tile([128, GPC], F32, tag="pv")

            bts, mrs = {}, {}

            def st1(g):
                j, gg = divmod(g, 2)
                base = j * PR + gg * KB * NPG
                p_bt = pbt.tile([128, NPG], F32, tag="bt")
                for kb in range(KB):
                    nc.tensor.matmul(
                        p_bt[:],
                        xp_sb[:, (g * KB + kb) * IN:(g * KB + kb + 1) * IN],
                        adj_all[:, base + kb * NPG: base + (kb + 1) * NPG],
                        start=(kb == 0), stop=(kb == KB - 1),
                    )
                bt_sb = bt_pool.tile([128, NPG], F16, tag="btsb")
                nc.vector.tensor_copy(out=bt_sb[:], in_=p_bt[:])
                bts[g] = bt_sb

            def st2(g):
                p_m = pm.tile([128, KB * H], F32, tag="m")
                for mb in range(KB):
                    nc.tensor.matmul(
                        p_m[:, mb * H:(mb + 1) * H],
                        bts[g][:, mb * 128:(mb + 1) * 128],
                        W1_sb,
                        start=True, stop=True,
                    )
                mr = mr_pool.tile([128, KB * H], F16, tag="mr")
                nc.scalar.activation(
                    out=mr[:], in_=p_m[:],
                    func=mybir.ActivationFunctionType.Relu,
                )
                mrs[g] = mr
                del bts[g]

            def pool(g):
                for mb in range(KB):
                    nc.tensor.matmul(
                        p_v[:, g:g + 1],
                        mrs[g][:, mb * H:(mb + 1) * H],
                        cdinv[:, g * KB + mb:g * KB + mb + 1],
                        start=(mb == 0), stop=(mb == KB - 1),
                    )
                del mrs[g]

            for it in range(GPC + 3):
                if it < GPC:
                    st1(it)
                if 2 <= it < GPC + 2:
                    st2(it - 2)
                if it >= 3:
                    pool(it - 3)

            # ---- fp16 MLP tail over virtT [H, GPC] ----
            virtT = consts.tile([H, GPC], F16, tag="virtT")
            nc.vector.tensor_copy(out=virtT[:], in_=p_v[:])
            p_t1 = ptail.tile([128, GPC], F32, tag="tail")
            nc.tensor.matmul(p_t1[:], wf16_sb[:, _C_VW1:_C_VW1 + H], virtT[:],
                             start=True, stop=True)
            t1 = consts.tile([H, GPC], F16, tag="t1")
            nc.scalar.activation(
                out=t1[:], in_=p_t1[:],
                func=mybir.ActivationFunctionType.Relu,
                bias=wf32_sb[:, _C_VB1:_C_VB1 + 1],
            )
            p_gf = ptail.tile([128, GPC], F32, tag="tail")
            nc.tensor.matmul(p_gf[:], wf16_sb[:, _C_VW2:_C_VW2 + H], t1[:],
                             start=True, stop=True)
            gf = consts.tile([H, GPC], F16, tag="gf")
            nc.scalar.activation(
                out=gf[:], in_=p_gf[:],
                func=mybir.ActivationFunctionType.Identity,
                bias=wf32_sb[:, _C_VB2:_C_VB2 + 1],
            )
            p_q1 = ptail.tile([128, GPC], F32, tag="tail")
            nc.tensor.matmul(p_q1[:], wf16_sb[:, _C_MW1:_C_MW1 + H], gf[:],
                             start=True, stop=True)
            q1 = consts.tile([H, GPC], F16, tag="q1")
            nc.scalar.activation(
                out=q1[:], in_=p_q1[:],
                func=mybir.ActivationFunctionType.Relu,
                bias=wf32_sb[:, _C_MB1:_C_MB1 + 1],
            )
            p_o = ptail.tile([OUT, GPC], F32, tag="tail")
            nc.tensor.matmul(p_o[:], wf16_sb[:, _C_MW2:_C_MW2 + OUT], q1[:],
                             start=True, stop=True)
            o_sb = consts.tile([OUT, GPC], F32, tag="osb")
            nc.scalar.activation(
                out=o_sb[:], in_=p_o[:],
                func=mybir.ActivationFunctionType.Identity,
                bias=wf32_sb[0:OUT, _C_MB2:_C_MB2 + 1],
            )
            nc.scalar.dma_start(out=outT[:], in_=o_sb[:])

    nc.finalize()
    return nc


def _build_program(with_bias: bool):
    """General per-core program (any edge_weights / biases)."""
    nc = bacc.Bacc("TRN2", target_bir_lowering=False)
    XDT = F16 if X_FP16 else F32

    # ---- DRAM I/O ----
    xsT = nc.dram_tensor("xsT", [IN, NS], XDT, kind="ExternalInput")          # dinv-scaled x, transposed
    W1 = nc.dram_tensor("W1", [IN, H], XDT, kind="ExternalInput")             # W_emb @ W_gcn
    # adjacency counts (+I), pre-arranged to SBUF layout, 2 graphs per row:
    # [j, p, gg*KB*NPG + kb*NPG + d]  (gg in {0,1}, graph = 2j+gg)
    adjT = nc.dram_tensor("adjT", [GPC // 2, 128, 2 * KB * NPG], F8, kind="ExternalInput")
    # dinv-scaled edge_weights, pre-arranged likewise
    ews = nc.dram_tensor("ews", [GPC // 2, 128, 2 * KB * V], F16, kind="ExternalInput")
    vW1 = nc.dram_tensor("vW1", [H, H], F32, kind="ExternalInput")
    vb1 = nc.dram_tensor("vb1", [H, 1], F32, kind="ExternalInput")
    vW2s = nc.dram_tensor("vW2s", [H, H], F32, kind="ExternalInput")          # vW2 / V
    vb2 = nc.dram_tensor("vb2", [H, 1], F32, kind="ExternalInput")
    mW1 = nc.dram_tensor("mW1", [H, H], F32, kind="ExternalInput")
    mb1 = nc.dram_tensor("mb1", [H, 1], F32, kind="ExternalInput")
    mW2 = nc.dram_tensor("mW2", [H, OUT], F32, kind="ExternalInput")
    mb2 = nc.dram_tensor("mb2", [OUT, 1], F32, kind="ExternalInput")
    if with_bias:
        biasL = nc.dram_tensor("biasL", [GPC, 2, NPG], F16, kind="ExternalInput")
        biasR = nc.dram_tensor("biasR", [2, H], F16, kind="ExternalInput")
    outT = nc.dram_tensor("outT", [OUT, GPC], F32, kind="ExternalOutput")

    with tile.TileContext(nc) as tc:
        with (
            tc.tile_pool(name="consts", bufs=1) as consts,
            tc.tile_pool(name="xchunk", bufs=4) as xchunk_pool,
            tc.tile_pool(name="upool", bufs=3) as u_pool,
            tc.tile_pool(name="adj", bufs=4) as adj_pool,
            tc.tile_pool(name="ewsp", bufs=4) as ews_pool,
            tc.tile_pool(name="mp", bufs=3) as m_pool,
            tc.tile_pool(name="blp", bufs=3) as bl_pool,
            tc.tile_pool(name="ph2", bufs=2, space="PSUM") as ph2,
            tc.tile_pool(name="pm", bufs=4, space="PSUM") as pm,
            tc.tile_pool(name="pv", bufs=1, space="PSUM") as pv,
            tc.tile_pool(name="pd", bufs=1, space="PSUM") as pd,
        ):
            # critical-path data first: graph pair 0's inputs, then W1
            xc0 = xchunk_pool.tile([IN, 2 * NPG], XDT, tag="xc")
            nc.sync.dma_start(out=xc0[:], in_=xsT[:, 0:2 * NPG])
            W1_sb = consts.tile([IN, H], XDT)
            nc.sync.dma_start(out=W1_sb[:], in_=W1[:])
            adj0 = adj_pool.tile([128, 2 * KB * NPG], F8, tag="adj")
            nc.sync.dma_start(out=adj0[:], in_=adjT[0])
            ews0 = ews_pool.tile([128, 2 * KB * V], F16, tag="ews")
            nc.sync.dma_start(out=ews0[:], in_=ews[0])

            vW1_sb = consts.tile([H, H], F32)
            nc.scalar.dma_start(out=vW1_sb[:], in_=vW1[:])
            vW2_sb = consts.tile([H, H], F32)
            nc.scalar.dma_start(out=vW2_sb[:], in_=vW2s[:])
            mW1_sb = consts.tile([H, H], F32)
            nc.scalar.dma_start(out=mW1_sb[:], in_=mW1[:])
            mW2_sb = consts.tile([H, OUT], F32)
            nc.scalar.dma_start(out=mW2_sb[:], in_=mW2[:])
            vb1_sb = consts.tile([H, 1], F32)
            nc.scalar.dma_start(out=vb1_sb[:], in_=vb1[:])
            vb2_sb = consts.tile([H, 1], F32)
            nc.scalar.dma_start(out=vb2_sb[:], in_=vb2[:])
            mb1_sb = consts.tile([H, 1], F32)
            nc.scalar.dma_start(out=mb1_sb[:], in_=mb1[:])
            mb2_sb = consts.tile([OUT, 1], F32)
            nc.scalar.dma_start(out=mb2_sb[:], in_=mb2[:])
            if with_bias:
                biasR_sb = consts.tile([2, H], F16)
                nc.scalar.dma_start(out=biasR_sb[:], in_=biasR[:])

            virtT = consts.tile([H, GPC * V], F32)  # virt^T, all graphs side by side
            t1 = consts.tile([H, GPC * V], F32)
            t1s = consts.tile([H, GPC], F32)

            def emit_embed(j):
                # u = (dinv*x) @ W1, cast fp16, for graph pair j (1024 nodes)
                if j == 0:
                    xc = xc0
                else:
                    xc = xchunk_pool.tile([IN, 2 * NPG], XDT, tag="xc")
                    nc.sync.dma_start(out=xc[:], in_=xsT[:, 2 * j * NPG:2 * (j + 1) * NPG])
                u_j = u_pool.tile([128, 2 * KB * H], F16, tag="u")
                for half in range(2):
                    p_h2 = ph2.tile([128, KB * H], F32, tag="ph2")
                    for kb in range(KB):
                        nc.tensor.matmul(
                            p_h2[:, kb * H:(kb + 1) * H],
                            xc[:, half * NPG + kb * 128: half * NPG + (kb + 1) * 128],
                            W1_sb[:],
                            start=True, stop=True,
                        )
                    nc.vector.tensor_copy(
                        out=u_j[:, half * KB * H:(half + 1) * KB * H], in_=p_h2[:])
                return u_j

            us = [emit_embed(0)]
            pending = []
            for g in range(GPC):
                j, gg = divmod(g, 2)
                if gg == 0:
                    if j + 1 < GPC // 2:
                        us.append(emit_embed(j + 1))
                    if j == 0:
                        adj_pair, ews_pair = adj0, ews0
                    else:
                        adj_pair = adj_pool.tile([128, 2 * KB * NPG], F8, tag="adj")
                        nc.sync.dma_start(out=adj_pair[:], in_=adjT[j])
                        ews_pair = ews_pool.tile([128, 2 * KB * V], F16, tag="ews")
                        nc.sync.dma_start(out=ews_pair[:], in_=ews[j])
                u_g = us[j][:, gg * KB * H:(gg + 1) * KB * H]
                adj_sb = adj_pair[:, gg * KB * NPG:(gg + 1) * KB * NPG]
                ews_sb = ews_pair[:, gg * KB * V:(gg + 1) * KB * V]
                if with_bias:
                    bl_sb = bl_pool.tile([2, NPG], F16, tag="bl")
                    nc.sync.dma_start(out=bl_sb[:], in_=biasL[g])

                m_sb = m_pool.tile([128, KB * H], F16, tag="m")
                for mb in range(KB):
                    p_m = pm.tile([128, H], F32, tag="pm")
                    if with_bias:
                        nc.tensor.matmul(
                            p_m[:], bl_sb[:, mb * 128:(mb + 1) * 128], biasR_sb[:],
                            start=True, stop=False,
                        )
                    for kb in range(KB):
                        nc.tensor.matmul(
                            p_m[:],
                            adj_sb[:, kb * NPG + mb * 128: kb * NPG + (mb + 1) * 128],
                            u_g[:, kb * H:(kb + 1) * H],
                            start=(kb == 0 and not with_bias),
                            stop=(kb == KB - 1),
                        )
                    nc.scalar.activation(
                        out=m_sb[:, mb * H:(mb + 1) * H], in_=p_m[:],
                        func=mybir.ActivationFunctionType.Relu,
                    )

                # ---- pooling (deferred by one graph so the relu is long done
                # by the time the PE reaches these matmuls) ----
                pending.append((g, m_sb, ews_sb))
                emit_g = g - 1 if g < GPC - 1 else None
                ready = [p for p in pending if p[0] == emit_g]
                if g == GPC - 1:
                    ready = list(pending)
                for eg, e_m, e_ews in ready:
                    p_v = pv.tile([128, V], F32, tag="pv")
                    for kb in range(KB):
                        nc.tensor.matmul(
                            p_v[:],
                            e_m[:, kb * H:(kb + 1) * H],
                            e_ews[:, kb * V:(kb + 1) * V],
                            start=(kb == 0), stop=(kb == KB - 1),
                        )
                    nc.vector.tensor_copy(out=virtT[:, eg * V:(eg + 1) * V], in_=p_v[:])
                    pending.remove((eg, e_m, e_ews))

                # ---- MLP first stage per quarter once its 4 graphs are emitted ----
                for q in range(4):
                    if g != (4 * q + 5 if q < 3 else GPC - 1):
                        continue
                    p_t1 = pd.tile([128, 256], F32, tag="pd")
                    nc.tensor.matmul(
                        p_t1[:], vW1_sb[:], virtT[:, q * 256:(q + 1) * 256],
                        start=True, stop=True,
                    )
                    nc.scalar.activation(
                        out=t1[:, q * 256:(q + 1) * 256], in_=p_t1[:],
                        func=mybir.ActivationFunctionType.Relu, bias=vb1_sb[:],
                    )
                    nc.vector.tensor_reduce(
                        out=t1s[:, q * 4:(q + 1) * 4],
                        in_=t1[:, q * 256:(q + 1) * 256]
                            .rearrange("p (g v) -> p g v", v=V),
                        axis=mybir.AxisListType.X, op=mybir.AluOpType.add,
                    )

            # ---- rest of the MLP tail ----
            p_gf = pd.tile([128, GPC], F32, tag="pd")
            nc.tensor.matmul(p_gf[:], vW2_sb[:], t1s[:], start=True, stop=True)
            gf = consts.tile([H, GPC], F32)
            nc.scalar.activation(
                out=gf[:], in_=p_gf[:],
                func=mybir.ActivationFunctionType.Identity, bias=vb2_sb[:],
            )
            p_q1 = pd.tile([128, GPC], F32, tag="pd")
            nc.tensor.matmul(p_q1[:], mW1_sb[:], gf[:], start=True, stop=True)
            q1 = consts.tile([H, GPC], F32)
            nc.scalar.activation(
                out=q1[:], in_=p_q1[:],
                func=mybir.ActivationFunctionType.Relu, bias=mb1_sb[:],
            )
            p_o = pd.tile([OUT, GPC], F32, tag="pd")
            nc.tensor.matmul(p_o[:], mW2_sb[:], q1[:], start=True, stop=True)
            o_sb = consts.tile([OUT, GPC], F32)
            nc.scalar.activation(
                out=o_sb[:], in_=p_o[:],
                func=mybir.ActivationFunctionType.Identity, bias=mb2_sb[:],
            )
            nc.sync.dma_start(out=outT[:], in_=o_sb[:])

    nc.finalize()
    return nc


def _reference_numpy(x, edge_index, W_emb, b_emb, W_gcn, b_gcn, edge_weights,
                     vW1, vb1, vW2, vb2, mW1, mb1, mW2, mb2):
    """Pure-numpy fallback (used only if graphs are not disjoint)."""
    src, dst = edge_index[0].astype(np.int64), edge_index[1].astype(np.int64)
    h = x @ W_emb + b_emb
    h2 = h @ W_gcn
    deg = np.bincount(dst, minlength=N).astype(np.float32) + 1.0
    dinv = 1.0 / np.sqrt(deg)
    m = np.zeros_like(h2)
    np.add.at(m, dst, h2[src] * (dinv[src] * dinv[dst])[:, None])
    m += h2 * (dinv * dinv)[:, None]
    m = np.maximum(m + b_gcn, 0.0)
    hg = m.reshape(G, NPG, -1)
    virt = np.einsum('gnv,gnh->gvh', edge_weights, hg)
    t1 = np.maximum(virt @ vW1 + vb1, 0.0) @ vW2 + vb2
    gf = t1.mean(axis=1)
    return np.maximum(gf @ mW1 + mb1, 0.0) @ mW2 + mb2


def kernel(x, edge_index, batch, W_emb, b_emb, W_gcn, b_gcn, edge_weights,
           vW1, vb1, vW2, vb2, mW1, mb1, mW2, mb2, _trace=False):
    x = np.asarray(x, dtype=np.float32)
    edge_index = np.asarray(edge_index, dtype=np.int32)
    W_emb = np.asarray(W_emb, dtype=np.float32)
    b_emb = np.asarray(b_emb, dtype=np.float32)
    W_gcn = np.asarray(W_gcn, dtype=np.float32)
    b_gcn = np.asarray(b_gcn, dtype=np.float32)
    edge_weights = np.asarray(edge_weights, dtype=np.float32)
    vW1, vb1 = np.asarray(vW1, np.float32), np.asarray(vb1, np.float32)
    vW2, vb2 = np.asarray(vW2, np.float32), np.asarray(vb2, np.float32)
    mW1, mb1 = np.asarray(mW1, np.float32), np.asarray(mb1, np.float32)
    mW2, mb2 = np.asarray(mW2, np.float32), np.asarray(mb2, np.float32)

    src = edge_index[0].astype(np.int64)
    dst = edge_index[1].astype(np.int64)
    if not np.array_equal(src // NPG, dst // NPG):
        # cross-graph edges: dense per-graph adjacency doesn't apply
        return _reference_numpy(x, edge_index, W_emb, b_emb, W_gcn, b_gcn,
                                edge_weights, vW1, vb1, vW2, vb2, mW1, mb1,
                                mW2, mb2).astype(np.float32)

    # ---- host prep ----
    deg = (np.bincount(dst, minlength=N) + 1).astype(np.float32)  # in-degree + self loop
    dinv = (1.0 / np.sqrt(deg)).astype(np.float32)

    # per-graph transposed adjacency counts (+ self loops), exact small ints in fp8e4
    gidx = src // NPG
    lin = (gidx * NPG + (src % NPG)) * NPG + (dst % NPG)
    counts = np.bincount(lin, minlength=G * NPG * NPG)
    adjT_all = counts.reshape(G, NPG, NPG).astype(np.float32)
    diag = np.arange(NPG)
    adjT_all[:, diag, diag] += np.float32(1.0)
    if adjT_all.max() > 16:  # not exactly representable in fp8e4
        return _reference_numpy(x, edge_index, W_emb, b_emb, W_gcn, b_gcn,
                                edge_weights, vW1, vb1, vW2, vb2, mW1, mb1,
                                mW2, mb2).astype(np.float32)
    adjT_all = adjT_all.astype(ml_dtypes.float8_e4m3)
    # SBUF layout: [g, p, kb*NPG + d], then merge graph pairs so each DMA is
    # one [128, contiguous] block covering 2 graphs
    adjT_sb_all = (
        adjT_all.reshape(G, KB, 128, NPG).transpose(0, 2, 1, 3).reshape(G, 128, KB * NPG)
    )
    adjT_sb_all = np.ascontiguousarray(
        adjT_sb_all.reshape(G // 2, 2, 128, KB * NPG).transpose(0, 2, 1, 3)
        .reshape(G // 2, 128, 2 * KB * NPG)
    )

    bvec = (b_emb @ W_gcn).astype(np.float32)
    with_bias = bool(np.any(bvec) or np.any(b_gcn))
    ew_col = edge_weights[:, :, 0]
    uniform = bool(np.all(edge_weights == ew_col[:, :, None]))
    W1h = (W_emb @ W_gcn).astype(np.float16)

    if uniform and not with_bias:
        # ---- fast path ----
        xs = (x * dinv[:, None]).astype(np.float16)      # fold D^-1/2_src into x
        # pooling weights: edge_weights column * dinv_dst
        cd = (ew_col * dinv.reshape(G, NPG)).astype(np.float16)  # [G, NPG]

        wf32_np = np.zeros((128, _C_W32), np.float32)
        wf32_np[:, _C_VB1] = vb1
        wf32_np[:, _C_VB2] = vb2
        wf32_np[:, _C_MB1] = mb1
        wf32_np[:OUT, _C_MB2] = mb2

        if "fast" not in _CACHE:
            _CACHE["fast"] = _build_fast()
        nc = _CACHE["fast"]

        # quad-merge adjacency pairs: [G//4, 128, 2 * 2*KB*NPG]
        adjT_quads = np.ascontiguousarray(
            adjT_sb_all.reshape(G // 4, 2, 128, 2 * KB * NPG).transpose(0, 2, 1, 3)
            .reshape(G // 4, 128, 4 * KB * NPG)
        )

        in_maps = []
        for c in range(N_CORES):
            xs_c = xs[c * NS:(c + 1) * NS]  # [8192, 128]
            xp_np = np.ascontiguousarray(
                xs_c.reshape(GPC * KB, 128, IN).transpose(1, 0, 2)
                .reshape(128, GPC * KB * IN)
            )
            cd_c = cd[c * GPC:(c + 1) * GPC]  # [GPC, NPG]
            cdp = np.ascontiguousarray(
                cd_c.reshape(GPC, KB, 128).transpose(2, 0, 1).reshape(128, GPC * KB)
            )
            wf16_np = np.zeros((128, _C_W16), np.float16)
            wf16_np[:, _C_W1:_C_W1 + IN] = W1h
            wf16_np[:, _C_CD:_C_CD + GPC * KB] = cdp
            wf16_np[:, _C_VW1:_C_VW1 + H] = vW1.astype(np.float16)
            wf16_np[:, _C_VW2:_C_VW2 + H] = vW2.astype(np.float16)
            wf16_np[:, _C_MW1:_C_MW1 + H] = mW1.astype(np.float16)
            wf16_np[:, _C_MW2:_C_MW2 + OUT] = mW2.astype(np.float16)
            qs = slice(c * GPC // 4, (c + 1) * GPC // 4)
            in_maps.append({
                "xp": xp_np,
                "adjT": adjT_quads[qs],
                "wf16": np.ascontiguousarray(wf16_np),
                "wf32": wf32_np,
            })
    else:
        # ---- general path ----
        xdt = np.float16 if X_FP16 else np.float32
        xs = (x * dinv[:, None])  # fold D^-1/2 into x rows
        xsT_np = np.ascontiguousarray(xs.T.astype(xdt))  # [IN, N]
        ews_all = (edge_weights * dinv.reshape(G, NPG, 1)).astype(np.float16)
        ews_sb_all = (
            ews_all.reshape(G, KB, 128, V).transpose(0, 2, 1, 3).reshape(G, 128, KB * V)
        )
        ews_sb_all = np.ascontiguousarray(
            ews_sb_all.reshape(G // 2, 2, 128, KB * V).transpose(0, 2, 1, 3)
            .reshape(G // 2, 128, 2 * KB * V)
        )

        vW2s_h = (vW2 / np.float32(V)).astype(np.float32)
        if with_bias:
            # m-psum bias = wvec ⊗ bvec + sqrt(deg) ⊗ b_gcn, with
            # wvec = (Adj+I) @ dinv per graph (host-computable rank-2 correction)
            dinv_g = dinv.reshape(G, NPG)
            wvec = np.einsum('gsd,gs->gd', adjT_all.astype(np.float32), dinv_g)
            sdeg = np.sqrt(deg).reshape(G, NPG)
            biasL_all = np.stack([wvec, sdeg], axis=1).astype(np.float16)  # [G, 2, NPG]
            biasR_np = np.stack([bvec, b_gcn], axis=0).astype(np.float16)  # [2, H]

        key = with_bias
        if key not in _CACHE:
            _CACHE[key] = _build_program(with_bias)
        nc = _CACHE[key]

        in_maps = []
        for c in range(N_CORES):
            gs = slice(c * GPC, (c + 1) * GPC)
            ps = slice(c * GPC // 2, (c + 1) * GPC // 2)
            im = {
                "xsT": np.ascontiguousarray(xsT_np[:, c * NS:(c + 1) * NS]),
                "W1": W1h if X_FP16 else (W_emb @ W_gcn).astype(np.float32),
                "adjT": adjT_sb_all[ps],
                "ews": ews_sb_all[ps],
                "vW1": vW1, "vb1": vb1.reshape(H, 1),
                "vW2s": vW2s_h, "vb2": vb2.reshape(H, 1),
                "mW1": mW1, "mb1": mb1.reshape(H, 1),
                "mW2": mW2, "mb2": mb2.reshape(OUT, 1),
            }
            if with_bias:
                im["biasL"] = np.ascontiguousarray(biasL_all[gs])
                im["biasR"] = biasR_np
            in_maps.append(im)

    res = run_bass_kernel_spmd(
        nc, in_maps, core_ids=list(range(N_CORES)), trace=_trace,
    )
    out = np.concatenate([res.results[c]["outT"].T for c in range(N_CORES)], axis=0)
    if _trace:
        kernel.last_exec_time_ns = res.exec_time_ns
        kernel.last_results = res
    return out.astype(np.float32)


# revision 10
# speedup vs baseline: 1.7824x; 1.0275x over previous
"""Trainium2 Bass kernel for BalancedIPRMPNN (GNN message passing).

Reference computation (G=128 disjoint graphs, NPG=512 nodes each, H=128):
    h2   = x @ (W_emb @ W_gcn) + b_emb @ W_gcn          # embedding+GCN linear folded
    m    = relu(D^-1/2 (Adj + I) D^-1/2 @ h2 + b_gcn)   # GCN propagate (per graph)
    virt = einsum('gnv,gnh->gvh', edge_weights, m)      # weighted pooling (V=64)
    t1   = relu(virt @ vW1 + vb1)
    gf   = mean_v(t1 @ vW2 + vb2)
    out  = relu(gf @ mW1 + mb1) @ mW2 + mb2             # [G, 10]

Strategy: data-parallel over graphs, 16 graphs per core on 8 cores.  Message
passing runs as a dense per-graph [512,512] adjacency matmul on the tensor
engine (integer counts exact in fp8e4); the symmetric degree normalization is
folded into x (rows pre-scaled by dinv_src) and the pooling weights
(pre-scaled by dinv_dst, legal since relu commutes with a positive scale when
b_gcn == 0).

Fast path (edge_weights uniform across the virtual-node axis, zero biases —
the shipped init): virt rows are v-independent, so the whole virtual-node
block collapses to a single weighted node-sum per graph.  Kernel chain per
graph g:
    BT_g  = sum_kb  xs_blk^T @ Adj_blk         (PE, adj is the fp8 moving op)
    m_g   = BT_g^T @ W1                        (PE, via SBUF-cast of BT)
    mr_g  = relu(m_g)                          (ScalarE, one [128,512] pass)
    virtT[:, g] = sum_mb mr_blk^T @ cdinv_blk  (PE, weighted pooling)
then a tiny fp32 MLP tail over virtT [H, 16].  All bulk DMAs are issued
up-front into persistent SBUF tiles on two queues so the wire never idles,
and a few dummy matmuls warm the PE clock gate during the initial DMA wait.

Non-uniform edge_weights or nonzero biases fall back to the general program
(same math as the reference, dense per-graph pooling); non-disjoint graphs or
adjacency counts not exactly representable in fp8 fall back to numpy.
"""

import ml_dtypes
import numpy as np

import concourse.mybir as mybir
import concourse.tile as tile
from concourse import bacc
from concourse.bass_utils import run_bass_kernel_spmd

# Problem constants (hardcoded per contract)
G, NPG, H, IN, OUT, V = 128, 512, 128, 128, 10, 64
N = G * NPG
N_CORES = 8
GPC = G // N_CORES          # graphs per core = 16
NS = GPC * NPG              # nodes per core  = 8192
KB = NPG // 128             # 4 k-blocks of 128 nodes per graph

F32 = mybir.dt.float32
F16 = mybir.dt.float16
F8 = mybir.dt.float8e4

X_FP16 = True               # ship x in fp16 (halves x DMA, 4x faster h2 matmuls)

_CACHE = {}

# fp16 const-pack column offsets (fast path): W1, cdinv, vW1, vW2, mW1, mW2
_C_W1, _C_CD = 0, 128
_C_VW1, _C_VW2, _C_MW1, _C_MW2 = 192, 320, 448, 576
_C_W16 = 586
# fp32 const-pack (biases): vb1, vb2, mb1, mb2
_C_VB1, _C_VB2, _C_MB1, _C_MB2 = 0, 1, 2, 3
_C_W32 = 4


def _build_fast():
    """Fast-path program: uniform edge_weights, zero biases."""
    nc = bacc.Bacc("TRN2", target_bir_lowering=False)

    PR = 2 * KB * NPG  # adjacency columns per graph pair (4096)

    xp = nc.dram_tensor("xp", [128, GPC * KB * IN], F16, kind="ExternalInput")
    # adjacency counts (+I), quad-merged: [q, p, jj*PR + gg*KB*NPG + kb*NPG + d]
    adjT = nc.dram_tensor("adjT", [GPC // 4, 128, 2 * PR], F8, kind="ExternalInput")
    # fp16 consts: W1 = W_emb @ W_gcn, cdinv pooling vecs, tail MLP weights
    wf16 = nc.dram_tensor("wf16", [128, _C_W16], F16, kind="ExternalInput")
    # fp32 consts: tail bias columns
    wf32 = nc.dram_tensor("wf32", [128, _C_W32], F32, kind="ExternalInput")
    outT = nc.dram_tensor("outT", [OUT, GPC], F32, kind="ExternalOutput")

    with tile.TileContext(nc) as tc:
        with (
            tc.tile_pool(name="consts", bufs=1) as consts,
            tc.tile_pool(name="btsb", bufs=3) as bt_pool,
            tc.tile_pool(name="mrel", bufs=4) as mr_pool,
            tc.tile_pool(name="pbt", bufs=2, space="PSUM") as pbt,
            tc.tile_pool(name="pm", bufs=2, space="PSUM") as pm,
            tc.tile_pool(name="pv", bufs=1, space="PSUM") as pv,
            tc.tile_pool(name="pwarm", bufs=1, space="PSUM") as pwarm,
            tc.tile_pool(name="ptail", bufs=2, space="PSUM") as ptail,
        ):
            # ---- bulk DMA plan -------------------------------------------
            # Everything rides the two HWDGE queues.  Emission order keeps
            # the 8 hardware DGE semaphores from being reused by a DMA whose
            # reuse-guard (wait for the prior use's consumers) could stall a
            # time-critical transfer: wf16 takes a never-reused sem; the
            # sync-queue bulk stream is ordered by consumption, and only its
            # tail entries (xp2, q67, xp3, wf32) land on reused sems, whose
            # guards (early graphs' matmuls) clear long before they matter.
            wf16_sb = consts.tile([128, _C_W16], F16)
            nc.scalar.dma_start(out=wf16_sb[:], in_=wf16[:])
            xp_sb = consts.tile([128, GPC * KB * IN], F16)
            adj_all = consts.tile([128, (GPC // 2) * PR], F8)
            wf32_sb = consts.tile([128, _C_W32], F32)

            XC = KB * IN * 4  # xp columns per 4-graph chunk (2048)

            def xp_dma(q):
                nc.sync.dma_start(out=xp_sb[:, q * XC:(q + 1) * XC],
                                  in_=xp[:, q * XC:(q + 1) * XC])

            def adj_dma(lo, hi):  # pairs [lo, hi) as one DMA (within one quad)
                assert lo // 2 == (hi - 1) // 2
                nc.sync.dma_start(
                    out=adj_all[:, lo * PR:hi * PR],
                    in_=adjT[lo // 2, :, (lo % 2) * PR:(lo % 2 + hi - lo) * PR],
                )

            GR = KB * NPG   # adjacency columns per graph (2048)
            GX = KB * IN    # xp columns per graph (512)

            def adj_dma_g(g):  # one graph's adjacency
                nc.sync.dma_start(
                    out=adj_all[:, g * GR:(g + 1) * GR],
                    in_=adjT[g // 4, :, (g % 4) * GR:(g % 4 + 1) * GR],
                )

            def xp_dma_g(lo, hi):  # graphs [lo, hi) of xp
                nc.sync.dma_start(out=xp_sb[:, lo * GX:hi * GX],
                                  in_=xp[:, lo * GX:hi * GX])

            # tiny first chunks so graph 0 can start ASAP, then big strides
            adj_dma_g(0)
            xp_dma_g(0, 2)
            adj_dma_g(1)
            xp_dma_g(2, 4)
            adj_dma(1, 2)
            adj_dma(2, 3)
            xp_dma(1)
            adj_dma(3, 4)
            adj_dma(4, 6)
            xp_dma(2)
            adj_dma(6, 8)
            xp_dma(3)
            nc.sync.dma_start(out=wf32_sb[:], in_=wf32[:])

            W1_sb = wf16_sb[:, _C_W1:_C_W1 + IN]
            cdinv = wf16_sb[:, _C_CD:_C_CD + GPC * KB]

            # ---- PE warm-up during the initial DMA wait ----
            scratch = consts.tile([128, NPG], F16)
            nc.gpsimd.memset(scratch[:], 0.0)
            p_w = pwarm.tile([128, NPG], F32, tag="warm")
            for _ in range(12):
                nc.tensor.matmul(p_w[:], scratch[:, 0:128], scratch[:],
                                 start=True, stop=True)

            p_v = pv.tile([128, GPC], F32, tag="pv")

            bts, mrs = {}, {}

            def st1(g):
                j, gg = divmod(g, 2)
                base = j * PR + gg * KB * NPG
                p_bt = pbt.tile([128, NPG], F32, tag="bt")
                for kb in range(KB):
                    nc.tensor.matmul(
                        p_bt[:],
                        xp_sb[:, (g * KB + kb) * IN:(g * KB + kb + 1) * IN],
                        adj_all[:, base + kb * NPG: base + (kb + 1) * NPG],
                        start=(kb == 0), stop=(kb == KB - 1),
                    )
                bt_sb = bt_pool.tile([128, NPG], F16, tag="btsb")
                nc.vector.tensor_copy(out=bt_sb[:], in_=p_bt[:])
                bts[g] = bt_sb

            def st2(g):
                p_m = pm.tile([128, KB * H], F32, tag="m")
                for mb in range(KB):
                    nc.tensor.matmul(
                        p_m[:, mb * H:(mb + 1) * H],
                        bts[g][:, mb * 128:(mb + 1) * 128],
                        W1_sb,
                        start=True, stop=True,
                    )
                mr = mr_pool.tile([128, KB * H], F16, tag="mr")
                nc.scalar.activation(
                    out=mr[:], in_=p_m[:],
                    func=mybir.ActivationFunctionType.Relu,
                )
                mrs[g] = mr
                del bts[g]

            def pool(g):
                for mb in range(KB):
                    nc.tensor.matmul(
                        p_v[:, g:g + 1],
                        mrs[g][:, mb * H:(mb + 1) * H],
                        cdinv[:, g * KB + mb:g * KB + mb + 1],
                        start=(mb == 0), stop=(mb == KB - 1),
                    )
                del mrs[g]

            for it in range(GPC + 4):
                if it < GPC:
                    st1(it)
                if 2 <= it < GPC + 2:
                    st2(it - 2)
                if it >= 4:
                    pool(it - 4)

            # ---- fp16 MLP tail over virtT [H, GPC] ----
            virtT = consts.tile([H, GPC], F16, tag="virtT")
            nc.vector.tensor_copy(out=virtT[:], in_=p_v[:])
            p_t1 = ptail.tile([128, GPC], F32, tag="tail")
            nc.tensor.matmul(p_t1[:], wf16_sb[:, _C_VW1:_C_VW1 + H], virtT[:],
                             start=True, stop=True)
            t1 = consts.tile([H, GPC], F16, tag="t1")
            nc.scalar.activation(
                out=t1[:], in_=p_t1[:],
                func=mybir.ActivationFunctionType.Relu,
                bias=wf32_sb[:, _C_VB1:_C_VB1 + 1],
            )
            p_gf = ptail.tile([128, GPC], F32, tag="tail")
            nc.tensor.matmul(p_gf[:], wf16_sb[:, _C_VW2:_C_VW2 + H], t1[:],
                             start=True, stop=True)
            gf = consts.tile([H, GPC], F16, tag="gf")
            nc.scalar.activation(
                out=gf[:], in_=p_gf[:],
                func=mybir.ActivationFunctionType.Identity,
                bias=wf32_sb[:, _C_VB2:_C_VB2 + 1],
            )
            p_q1 = ptail.tile([128, GPC], F32, tag="tail")
            nc.tensor.matmul(p_q1[:], wf16_sb[:, _C_MW1:_C_MW1 + H], gf[:],
                             start=True, stop=True)
            q1 = consts.tile([H, GPC], F16, tag="q1")
            nc.scalar.activation(
                out=q1[:], in_=p_q1[:],
                func=mybir.ActivationFunctionType.Relu,
                bias=wf32_sb[:, _C_MB1:_C_MB1 + 1],
            )
            p_o = ptail.tile([OUT, GPC], F32, tag="tail")
            nc.tensor.matmul(p_o[:], wf16_sb[:, _C_MW2:_C_MW2 + OUT], q1[:],
                             start=True, stop=True)
            o_sb = consts.tile([OUT, GPC], F32, tag="osb")
            nc.scalar.activation(
                out=o_sb[:], in_=p_o[:],
                func=mybir.ActivationFunctionType.Identity,
                bias=wf32_sb[0:OUT, _C_MB2:_C_MB2 + 1],
            )
            nc.scalar.dma_start(out=outT[:], in_=o_sb[:])

    nc.finalize()
    return nc


def _build_program(with_bias: bool):
    """General per-core program (any edge_weights / biases)."""
    nc = bacc.Bacc("TRN2", target_bir_lowering=False)
    XDT = F16 if X_FP16 else F32

    # ---- DRAM I/O ----
    xsT = nc.dram_tensor("xsT", [IN, NS], XDT, kind="ExternalInput")          # dinv-scaled x, transposed
    W1 = nc.dram_tensor("W1", [IN, H], XDT, kind="ExternalInput")             # W_emb @ W_gcn
    # adjacency counts (+I), pre-arranged to SBUF layout, 2 graphs per row:
    # [j, p, gg*KB*NPG + kb*NPG + d]  (gg in {0,1}, graph = 2j+gg)
    adjT = nc.dram_tensor("adjT", [GPC // 2, 128, 2 * KB * NPG], F8, kind="ExternalInput")
    # dinv-scaled edge_weights, pre-arranged likewise
    ews = nc.dram_tensor("ews", [GPC // 2, 128, 2 * KB * V], F16, kind="ExternalInput")
    vW1 = nc.dram_tensor("vW1", [H, H], F32, kind="ExternalInput")
    vb1 = nc.dram_tensor("vb1", [H, 1], F32, kind="ExternalInput")
    vW2s = nc.dram_tensor("vW2s", [H, H], F32, kind="ExternalInput")          # vW2 / V
    vb2 = nc.dram_tensor("vb2", [H, 1], F32, kind="ExternalInput")
    mW1 = nc.dram_tensor("mW1", [H, H], F32, kind="ExternalInput")
    mb1 = nc.dram_tensor("mb1", [H, 1], F32, kind="ExternalInput")
    mW2 = nc.dram_tensor("mW2", [H, OUT], F32, kind="ExternalInput")
    mb2 = nc.dram_tensor("mb2", [OUT, 1], F32, kind="ExternalInput")
    if with_bias:
        biasL = nc.dram_tensor("biasL", [GPC, 2, NPG], F16, kind="ExternalInput")
        biasR = nc.dram_tensor("biasR", [2, H], F16, kind="ExternalInput")
    outT = nc.dram_tensor("outT", [OUT, GPC], F32, kind="ExternalOutput")

    with tile.TileContext(nc) as tc:
        with (
            tc.tile_pool(name="consts", bufs=1) as consts,
            tc.tile_pool(name="xchunk", bufs=4) as xchunk_pool,
            tc.tile_pool(name="upool", bufs=3) as u_pool,
            tc.tile_pool(name="adj", bufs=4) as adj_pool,
            tc.tile_pool(name="ewsp", bufs=4) as ews_pool,
            tc.tile_pool(name="mp", bufs=3) as m_pool,
            tc.tile_pool(name="blp", bufs=3) as bl_pool,
            tc.tile_pool(name="ph2", bufs=2, space="PSUM") as ph2,
            tc.tile_pool(name="pm", bufs=4, space="PSUM") as pm,
            tc.tile_pool(name="pv", bufs=1, space="PSUM") as pv,
            tc.tile_pool(name="pd", bufs=1, space="PSUM") as pd,
        ):
            # critical-path data first: graph pair 0's inputs, then W1
            xc0 = xchunk_pool.tile([IN, 2 * NPG], XDT, tag="xc")
            nc.sync.dma_start(out=xc0[:], in_=xsT[:, 0:2 * NPG])
            W1_sb = consts.tile([IN, H], XDT)
            nc.sync.dma_start(out=W1_sb[:], in_=W1[:])
            adj0 = adj_pool.tile([128, 2 * KB * NPG], F8, tag="adj")
            nc.sync.dma_start(out=adj0[:], in_=adjT[0])
            ews0 = ews_pool.tile([128, 2 * KB * V], F16, tag="ews")
            nc.sync.dma_start(out=ews0[:], in_=ews[0])

            vW1_sb = consts.tile([H, H], F32)
            nc.scalar.dma_start(out=vW1_sb[:], in_=vW1[:])
            vW2_sb = consts.tile([H, H], F32)
            nc.scalar.dma_start(out=vW2_sb[:], in_=vW2s[:])
            mW1_sb = consts.tile([H, H], F32)
            nc.scalar.dma_start(out=mW1_sb[:], in_=mW1[:])
            mW2_sb = consts.tile([H, OUT], F32)
            nc.scalar.dma_start(out=mW2_sb[:], in_=mW2[:])
            vb1_sb = consts.tile([H, 1], F32)
            nc.scalar.dma_start(out=vb1_sb[:], in_=vb1[:])
            vb2_sb = consts.tile([H, 1], F32)
            nc.scalar.dma_start(out=vb2_sb[:], in_=vb2[:])
            mb1_sb = consts.tile([H, 1], F32)
            nc.scalar.dma_start(out=mb1_sb[:], in_=mb1[:])
            mb2_sb = consts.tile([OUT, 1], F32)
            nc.scalar.dma_start(out=mb2_sb[:], in_=mb2[:])
            if with_bias:
                biasR_sb = consts.tile([2, H], F16)
                nc.scalar.dma_start(out=biasR_sb[:], in_=biasR[:])

            virtT = consts.tile([H, GPC * V], F32)  # virt^T, all graphs side by side
            t1 = consts.tile([H, GPC * V], F32)
            t1s = consts.tile([H, GPC], F32)

            def emit_embed(j):
                # u = (dinv*x) @ W1, cast fp16, for graph pair j (1024 nodes)
                if j == 0:
                    xc = xc0
                else:
                    xc = xchunk_pool.tile([IN, 2 * NPG], XDT, tag="xc")
                    nc.sync.dma_start(out=xc[:], in_=xsT[:, 2 * j * NPG:2 * (j + 1) * NPG])
                u_j = u_pool.tile([128, 2 * KB * H], F16, tag="u")
                for half in range(2):
                    p_h2 = ph2.tile([128, KB * H], F32, tag="ph2")
                    for kb in range(KB):
                        nc.tensor.matmul(
                            p_h2[:, kb * H:(kb + 1) * H],
                            xc[:, half * NPG + kb * 128: half * NPG + (kb + 1) * 128],
                            W1_sb[:],
                            start=True, stop=True,
                        )
                    nc.vector.tensor_copy(
                        out=u_j[:, half * KB * H:(half + 1) * KB * H], in_=p_h2[:])
                return u_j

            us = [emit_embed(0)]
            pending = []
            for g in range(GPC):
                j, gg = divmod(g, 2)
                if gg == 0:
                    if j + 1 < GPC // 2:
                        us.append(emit_embed(j + 1))
                    if j == 0:
                        adj_pair, ews_pair = adj0, ews0
                    else:
                        adj_pair = adj_pool.tile([128, 2 * KB * NPG], F8, tag="adj")
                        nc.sync.dma_start(out=adj_pair[:], in_=adjT[j])
                        ews_pair = ews_pool.tile([128, 2 * KB * V], F16, tag="ews")
                        nc.sync.dma_start(out=ews_pair[:], in_=ews[j])
                u_g = us[j][:, gg * KB * H:(gg + 1) * KB * H]
                adj_sb = adj_pair[:, gg * KB * NPG:(gg + 1) * KB * NPG]
                ews_sb = ews_pair[:, gg * KB * V:(gg + 1) * KB * V]
                if with_bias:
                    bl_sb = bl_pool.tile([2, NPG], F16, tag="bl")
                    nc.sync.dma_start(out=bl_sb[:], in_=biasL[g])

                m_sb = m_pool.tile([128, KB * H], F16, tag="m")
                for mb in range(KB):
                    p_m = pm.tile([128, H], F32, tag="pm")
                    if with_bias:
                        nc.tensor.matmul(
                            p_m[:], bl_sb[:, mb * 128:(mb + 1) * 128], biasR_sb[:],
                            start=True, stop=False,
                        )
                    for kb in range(KB):
                        nc.tensor.matmul(
                            p_m[:],
                            adj_sb[:, kb * NPG + mb * 128: kb * NPG + (mb + 1) * 128],
                            u_g[:, kb * H:(kb + 1) * H],
                            start=(kb == 0 and not with_bias),
                            stop=(kb == KB - 1),
                        )
                    nc.scalar.activation(
                        out=m_sb[:, mb * H:(mb + 1) * H], in_=p_m[:],
                        func=mybir.ActivationFunctionType.Relu,
                    )

                # ---- pooling (deferred by one graph so the relu is long done
                # by the time the PE reaches these matmuls) ----
                pending.append((g, m_sb, ews_sb))
                emit_g = g - 1 if g < GPC - 1 else None
                ready = [p for p in pending if p[0] == emit_g]
                if g == GPC - 1:
                    ready = list(pending)
                for eg, e_m, e_ews in ready:
                    p_v = pv.tile([128, V], F32, tag="pv")
                    for kb in range(KB):
                        nc.tensor.matmul(
                            p_v[:],
                            e_m[:, kb * H:(kb + 1) * H],
                            e_ews[:, kb * V:(kb + 1) * V],
                            start=(kb == 0), stop=(kb == KB - 1),
                        )
                    nc.vector.tensor_copy(out=virtT[:, eg * V:(eg + 1) * V], in_=p_v[:])
                    pending.remove((eg, e_m, e_ews))

                # ---- MLP first stage per quarter once its 4 graphs are emitted ----
                for q in range(4):
                    if g != (4 * q + 5 if q < 3 else GPC - 1):
                        continue
                    p_t1 = pd.tile([128, 256], F32, tag="pd")
                    nc.tensor.matmul(
                        p_t1[:], vW1_sb[:], virtT[:, q * 256:(q + 1) * 256],
                        start=True, stop=True,
                    )
                    nc.scalar.activation(
                        out=t1[:, q * 256:(q + 1) * 256], in_=p_t1[:],
                        func=mybir.ActivationFunctionType.Relu, bias=vb1_sb[:],
                    )
                    nc.vector.tensor_reduce(
                        out=t1s[:, q * 4:(q + 1) * 4],
                        in_=t1[:, q * 256:(q + 1) * 256]
                            .rearrange("p (g v) -> p g v", v=V),
                        axis=mybir.AxisListType.X, op=mybir.AluOpType.add,
                    )

            # ---- rest of the MLP tail ----
            p_gf = pd.tile([128, GPC], F32, tag="pd")
            nc.tensor.matmul(p_gf[:], vW2_sb[:], t1s[:], start=True, stop=True)
            gf = consts.tile([H, GPC], F32)
            nc.scalar.activation(
                out=gf[:], in_=p_gf[:],
                func=mybir.ActivationFunctionType.Identity, bias=vb2_sb[:],
            )
            p_q1 = pd.tile([128, GPC], F32, tag="pd")
            nc.tensor.matmul(p_q1[:], mW1_sb[:], gf[:], start=True, stop=True)
            q1 = consts.tile([H, GPC], F32)
            nc.scalar.activation(
                out=q1[:], in_=p_q1[:],
                func=mybir.ActivationFunctionType.Relu, bias=mb1_sb[:],
            )
            p_o = pd.tile([OUT, GPC], F32, tag="pd")
            nc.tensor.matmul(p_o[:], mW2_sb[:], q1[:], start=True, stop=True)
            o_sb = consts.tile([OUT, GPC], F32)
            nc.scalar.activation(
                out=o_sb[:], in_=p_o[:],
                func=mybir.ActivationFunctionType.Identity, bias=mb2_sb[:],
            )
            nc.sync.dma_start(out=outT[:], in_=o_sb[:])

    nc.finalize()
    return nc


def _reference_numpy(x, edge_index, W_emb, b_emb, W_gcn, b_gcn, edge_weights,
                     vW1, vb1, vW2, vb2, mW1, mb1, mW2, mb2):
    """Pure-numpy fallback (used only if graphs are not disjoint)."""
    src, dst = edge_index[0].astype(np.int64), edge_index[1].astype(np.int64)
    h = x @ W_emb + b_emb
    h2 = h @ W_gcn
    deg = np.bincount(dst, minlength=N).astype(np.float32) + 1.0
    dinv = 1.0 / np.sqrt(deg)
    m = np.zeros_like(h2)
    np.add.at(m, dst, h2[src] * (dinv[src] * dinv[dst])[:, None])
    m += h2 * (dinv * dinv)[:, None]
    m = np.maximum(m + b_gcn, 0.0)
    hg = m.reshape(G, NPG, -1)
    virt = np.einsum('gnv,gnh->gvh', edge_weights, hg)
    t1 = np.maximum(virt @ vW1 + vb1, 0.0) @ vW2 + vb2
    gf = t1.mean(axis=1)
    return np.maximum(gf @ mW1 + mb1, 0.0) @ mW2 + mb2


def kernel(x, edge_index, batch, W_emb, b_emb, W_gcn, b_gcn, edge_weights,
           vW1, vb1, vW2, vb2, mW1, mb1, mW2, mb2, _trace=False):
    x = np.asarray(x, dtype=np.float32)
    edge_index = np.asarray(edge_index, dtype=np.int32)
    W_emb = np.asarray(W_emb, dtype=np.float32)
    b_emb = np.asarray(b_emb, dtype=np.float32)
    W_gcn = np.asarray(W_gcn, dtype=np.float32)
    b_gcn = np.asarray(b_gcn, dtype=np.float32)
    edge_weights = np.asarray(edge_weights, dtype=np.float32)
    vW1, vb1 = np.asarray(vW1, np.float32), np.asarray(vb1, np.float32)
    vW2, vb2 = np.asarray(vW2, np.float32), np.asarray(vb2, np.float32)
    mW1, mb1 = np.asarray(mW1, np.float32), np.asarray(mb1, np.float32)
    mW2, mb2 = np.asarray(mW2, np.float32), np.asarray(mb2, np.float32)

    src = edge_index[0].astype(np.int64)
    dst = edge_index[1].astype(np.int64)
    if not np.array_equal(src // NPG, dst // NPG):
        # cross-graph edges: dense per-graph adjacency doesn't apply
        return _reference_numpy(x, edge_index, W_emb, b_emb, W_gcn, b_gcn,
                                edge_weights, vW1, vb1, vW2, vb2, mW1, mb1,
                                mW2, mb2).astype(np.float32)

    # ---- host prep ----
    deg = (np.bincount(dst, minlength=N) + 1).astype(np.float32)  # in-degree + self loop
    dinv = (1.0 / np.sqrt(deg)).astype(np.float32)

    # per-graph transposed adjacency counts (+ self loops), exact small ints in fp8e4
    gidx = src // NPG
    lin = (gidx * NPG + (src % NPG)) * NPG + (dst % NPG)
    counts = np.bincount(lin, minlength=G * NPG * NPG)
    adjT_all = counts.reshape(G, NPG, NPG).astype(np.float32)
    diag = np.arange(NPG)
    adjT_all[:, diag, diag] += np.float32(1.0)
    if adjT_all.max() > 16:  # not exactly representable in fp8e4
        return _reference_numpy(x, edge_index, W_emb, b_emb, W_gcn, b_gcn,
                                edge_weights, vW1, vb1, vW2, vb2, mW1, mb1,
                                mW2, mb2).astype(np.float32)
    adjT_all = adjT_all.astype(ml_dtypes.float8_e4m3)
    # SBUF layout: [g, p, kb*NPG + d], then merge graph pairs so each DMA is
    # one [128, contiguous] block covering 2 graphs
    adjT_sb_all = (
        adjT_all.reshape(G, KB, 128, NPG).transpose(0, 2, 1, 3).reshape(G, 128, KB * NPG)
    )
    adjT_sb_all = np.ascontiguousarray(
        adjT_sb_all.reshape(G // 2, 2, 128, KB * NPG).transpose(0, 2, 1, 3)
        .reshape(G // 2, 128, 2 * KB * NPG)
    )

    bvec = (b_emb @ W_gcn).astype(np.float32)
    with_bias = bool(np.any(bvec) or np.any(b_gcn))
    ew_col = edge_weights[:, :, 0]
    uniform = bool(np.all(edge_weights == ew_col[:, :, None]))
    W1h = (W_emb @ W_gcn).astype(np.float16)

    if uniform and not with_bias:
        # ---- fast path ----
        xs = (x * dinv[:, None]).astype(np.float16)      # fold D^-1/2_src into x
        # pooling weights: edge_weights column * dinv_dst
        cd = (ew_col * dinv.reshape(G, NPG)).astype(np.float16)  # [G, NPG]

        wf32_np = np.zeros((128, _C_W32), np.float32)
        wf32_np[:, _C_VB1] = vb1
        wf32_np[:, _C_VB2] = vb2
        wf32_np[:, _C_MB1] = mb1
        wf32_np[:OUT, _C_MB2] = mb2

        if "fast" not in _CACHE:
            _CACHE["fast"] = _build_fast()
        nc = _CACHE["fast"]

        # quad-merge adjacency pairs: [G//4, 128, 2 * 2*KB*NPG]
        adjT_quads = np.ascontiguousarray(
            adjT_sb_all.reshape(G // 4, 2, 128, 2 * KB * NPG).transpose(0, 2, 1, 3)
            .reshape(G // 4, 128, 4 * KB * NPG)
        )

        in_maps = []
        for c in range(N_CORES):
            xs_c = xs[c * NS:(c + 1) * NS]  # [8192, 128]
            xp_np = np.ascontiguousarray(
                xs_c.reshape(GPC * KB, 128, IN).transpose(1, 0, 2)
                .reshape(128, GPC * KB * IN)
            )
            cd_c = cd[c * GPC:(c + 1) * GPC]  # [GPC, NPG]
            cdp = np.ascontiguousarray(
                cd_c.reshape(GPC, KB, 128).transpose(2, 0, 1).reshape(128, GPC * KB)
            )
            wf16_np = np.zeros((128, _C_W16), np.float16)
            wf16_np[:, _C_W1:_C_W1 + IN] = W1h
            wf16_np[:, _C_CD:_C_CD + GPC * KB] = cdp
            wf16_np[:, _C_VW1:_C_VW1 + H] = vW1.astype(np.float16)
            wf16_np[:, _C_VW2:_C_VW2 + H] = vW2.astype(np.float16)
            wf16_np[:, _C_MW1:_C_MW1 + H] = mW1.astype(np.float16)
            wf16_np[:, _C_MW2:_C_MW2 + OUT] = mW2.astype(np.float16)
            qs = slice(c * GPC // 4, (c + 1) * GPC // 4)
            in_maps.append({
                "xp": xp_np,
                "adjT": adjT_quads[qs],
                "wf16": np.ascontiguousarray(wf16_np),
                "wf32": wf32_np,
            })
    else:
        # ---- general path ----
        xdt = np.float16 if X_FP16 else np.float32
        xs = (x * dinv[:, None])  # fold D^-1/2 into x rows
        xsT_np = np.ascontiguousarray(xs.T.astype(xdt))  # [IN, N]
        ews_all = (edge_weights * dinv.reshape(G, NPG, 1)).astype(np.float16)
        ews_sb_all = (
            ews_all.reshape(G, KB, 128, V).transpose(0, 2, 1, 3).reshape(G, 128, KB * V)
        )
        ews_sb_all = np.ascontiguousarray(
            ews_sb_all.reshape(G // 2, 2, 128, KB * V).transpose(0, 2, 1, 3)
            .reshape(G // 2, 128, 2 * KB * V)
        )

        vW2s_h = (vW2 / np.float32(V)).astype(np.float32)
        if with_bias:
            # m-psum bias = wvec ⊗ bvec + sqrt(deg) ⊗ b_gcn, with
            # wvec = (Adj+I) @ dinv per graph (host-computable rank-2 correction)
            dinv_g = dinv.reshape(G, NPG)
            wvec = np.einsum('gsd,gs->gd', adjT_all.astype(np.float32), dinv_g)
            sdeg = np.sqrt(deg).reshape(G, NPG)
            biasL_all = np.stack([wvec, sdeg], axis=1).astype(np.float16)  # [G, 2, NPG]
            biasR_np = np.stack([bvec, b_gcn], axis=0).astype(np.float16)  # [2, H]

        key = with_bias
        if key not in _CACHE:
            _CACHE[key] = _build_program(with_bias)
        nc = _CACHE[key]

        in_maps = []
        for c in range(N_CORES):
            gs = slice(c * GPC, (c + 1) * GPC)
            ps = slice(c * GPC // 2, (c + 1) * GPC // 2)
            im = {
                "xsT": np.ascontiguousarray(xsT_np[:, c * NS:(c + 1) * NS]),
                "W1": W1h if X_FP16 else (W_emb @ W_gcn).astype(np.float32),
                "adjT": adjT_sb_all[ps],
                "ews": ews_sb_all[ps],
                "vW1": vW1, "vb1": vb1.reshape(H, 1),
                "vW2s": vW2s_h, "vb2": vb2.reshape(H, 1),
                "mW1": mW1, "mb1": mb1.reshape(H, 1),
                "mW2": mW2, "mb2": mb2.reshape(OUT, 1),
            }
            if with_bias:
                im["biasL"] = np.ascontiguousarray(biasL_all[gs])
                im["biasR"] = biasR_np
            in_maps.append(im)

    res = run_bass_kernel_spmd(
        nc, in_maps, core_ids=list(range(N_CORES)), trace=_trace,
    )
    out = np.concatenate([res.results[c]["outT"].T for c in range(N_CORES)], axis=0)
    if _trace:
        kernel.last_exec_time_ns = res.exec_time_ns
        kernel.last_results = res
    return out.astype(np.float32)
